# revision 1
# baseline (speedup 1.0000x reference)
"""AGF attention (graph-filter attention) distributed Bass kernel for 8 TRN2 cores.

Sharding: batch x head-pair (data + head parallel). Core i handles batch
b = i//4 and heads {2*(i%4), 2*(i%4)+1}. Each core computes its partial
output projection (summed over its 2 heads); a bf16 ReduceScatter over the
4 cores of each batch produces row shards of that batch's [N, D] output,
which the host concatenates.

v2 design (per core):
  - P = exp(S^T/8) for the CURRENT head is kept RESIDENT in SBUF
    ([128, NC, N] fp8 = 128 KB/partition) -- no HBM spill/reload. The three
    graph-filter applications read P straight from SBUF.
  - Production: S^T chunks via bf16 matmuls (KT chunk stationary) into
    [128, 2048] PSUM tiles (4 banks x 2 bufs), exp'd by ACT directly into
    P_sb as fp8. ACT is the kernel's bottleneck engine (~250 us of exp).
  - Applications: fp8 DoubleRow matmuls, stationary [t*16 | 1] (TW=80),
    moving P pairs, accumulate u^T = [16*A_u t | r]^T in a [80, NB, 512]
    PSUM tile (8 banks). u^T -> bf16 -> PE-transposed back to natural
    layout in packs of 8 chunks per PSUM bank; epilogue (normalize by 1/r,
    t-requantize to fp8, y accumulation) is BATCHED into a handful of
    full-size DVE instructions using stride-0 broadcast APs.
  - Head 1's Q^T/K^T are produced in setup and spilled to DRAM (bf16),
    reloaded into the same SBUF tiles after head 0's production.
  - Output projection: y (natural, f32, both heads) -> PE transpose ->
    po PSUM accumulates BOTH heads -> bf16 -> ReduceScatter over the
    4-core batch group.
"""

import numpy as np

import concourse.mybir as mybir
import concourse.tile as tile
from concourse import bacc
from concourse.bass import ds
from concourse.masks import make_identity

dt = mybir.dt
F32 = dt.float32
BF16 = dt.bfloat16
FP8 = dt.float8e4
AF = mybir.ActivationFunctionType
ALU = mybir.AluOpType
AX = mybir.AxisListType
DR = mybir.MatmulPerfMode.DoubleRow

D = 256      # model dim
DH = 64      # head dim
HPC = 2      # heads per core
LN_EPS = 1e-5
SM_SCALE = 0.125  # 1/sqrt(DH)
TW = 80      # t tile width: 64 t cols + 1 rowsum + pad to 16 (DoubleRow)
T_SCALE = 16.0


def build_kernel(nc, N=4096, replica_groups=((0, 1, 2, 3), (4, 5, 6, 7)),
                 p_dtype=FP8, collective=True):
    NC = N // 128          # 128-row chunks
    NB = N // 512          # 512-col blocks
    KD = D // 128          # 128-deep contraction chunks of the model dim
    replica_groups = [list(g) for g in replica_groups]

    G = len(replica_groups[0]) if collective else 4
    x = nc.dram_tensor("x", [N, D], F32, kind="ExternalInput")
    wq_d = nc.dram_tensor("wq", [D, HPC * DH], F32, kind="ExternalInput")
    wk_d = nc.dram_tensor("wk", [D, HPC * DH], F32, kind="ExternalInput")
    wv_d = nc.dram_tensor("wv", [D, HPC * DH], F32, kind="ExternalInput")
    wo_d = nc.dram_tensor("wo", [HPC * DH, D], F32, kind="ExternalInput")
    gamma_d = nc.dram_tensor("gamma2", [1, HPC * DH], F32, kind="ExternalInput")
    beta_d = nc.dram_tensor("beta2", [1, HPC * DH], F32, kind="ExternalInput")
    cf_d = nc.dram_tensor("cf", [1, HPC * 4], F32, kind="ExternalInput")
    out_d = nc.dram_tensor("out", [N // G, D], BF16, kind="ExternalOutput")

    with tile.TileContext(nc) as tc:
        _body(nc, tc, N, NC, NB, KD, replica_groups,
              x, wq_d, wk_d, wv_d, wo_d, gamma_d, beta_d, cf_d, out_d,
              collective)
    return nc


def _body(nc, tc, N, NC, NB, KD, replica_groups,
          x, wq_d, wk_d, wv_d, wo_d, gamma_d, beta_d, cf_d, out_d,
          collective=True):
    with (
        tc.tile_pool(name="persist", bufs=1) as pp,
        tc.tile_pool(name="dram", bufs=1, space="DRAM") as dram,
    ):
        # ---------- constants ----------
        ident = pp.tile([128, 128], F32)
        make_identity(nc, ident)
        ident_b = pp.tile([128, 128], BF16)
        nc.vector.tensor_copy(ident_b[:], ident[:])

        c1 = pp.tile([1, 8], F32)
        nc.sync.dma_start(c1[:], cf_d.ap())
        cbc = pp.tile([128, 8], F32)
        nc.gpsimd.partition_broadcast(cbc[:], c1[:])
        cbc16 = pp.tile([128, 8], F32)
        nc.vector.tensor_scalar_mul(cbc16[:], cbc[:], 1.0 / T_SCALE)

        # ---------- persistent work tiles ----------
        QT = pp.tile([64, N], BF16)     # current head's Q^T
        KT = pp.tile([64, N], BF16)     # current head's K^T
        vones = pp.tile([128, NC, HPC, TW], FP8)   # [v_ln | 1] app-1 lhs
        nc.vector.memset(vones[:, :, :, 64:TW], 1.0)
        vln = pp.tile([128, NC, HPC, 64], BF16)    # LayerNorm'd v
        tq = pp.tile([128, NC, TW], FP8)           # [16*t_k | 1] app lhs
        nc.vector.memset(tq[:, :, 64:TW], 1.0)
        uSt = pp.tile([128, NC, TW], BF16)         # transposed-back u chunks
        rinv = pp.tile([128, NC, 1], F32)          # 1/rowsum per (n-chunk)
        yt = pp.tile([128, NC, HPC, 64], F32)      # y, natural, both heads

        wo_s = pp.tile([64, HPC, 256], BF16)
        qk_dr = dram.tile([2, 64, N], BF16)        # head-1 Q^T/K^T spill
        bounce_in = dram.tile([N, D], BF16)

        # ================= setup =================
        with tc.tile_pool(name="setup", bufs=1) as sp:
            # weights -> SBUF bf16
            wst = sp.tile([128, 3, KD, 128], F32)
            nc.sync.dma_start(
                wst[:, 0], wq_d.ap().rearrange("(o p) m -> p o m", p=128))
            nc.sync.dma_start(
                wst[:, 1], wk_d.ap().rearrange("(o p) m -> p o m", p=128))
            nc.scalar.dma_start(
                wst[:, 2], wv_d.ap().rearrange("(o p) m -> p o m", p=128))
            wsb = sp.tile([128, 3, KD, 128], BF16)
            nc.vector.tensor_copy(wsb[:], wst[:])
            wq_s, wk_s, wv_s = wsb[:, 0], wsb[:, 1], wsb[:, 2]

            wo_f = sp.tile([64, HPC, 256], F32)
            nc.scalar.dma_start(
                wo_f[:], wo_d.ap().rearrange("(h d) m -> d h m", h=HPC))
            nc.vector.tensor_copy(wo_s[:], wo_f[:])

            g1 = sp.tile([1, 128], F32)
            nc.sync.dma_start(g1[:], gamma_d.ap())
            gbc = sp.tile([128, 1, HPC, 64], F32)
            nc.gpsimd.partition_broadcast(
                gbc.rearrange("p o a b -> p (o a b)"), g1[:])
            b1 = sp.tile([1, 128], F32)
            nc.sync.dma_start(b1[:], beta_d.ap())
            bbc = sp.tile([128, 1, HPC, 64], F32)
            nc.gpsimd.partition_broadcast(
                bbc.rearrange("p o a b -> p (o a b)"), b1[:])

            # x -> x^T via PE transposes (low latency; no DRAM bounce)
            xT = sp.tile([128, KD, N], BF16)
            with (
                tc.tile_pool(name="xsetup", bufs=3) as xp,
                tc.tile_pool(name="xt_psum", bufs=2, space="PSUM") as xtp,
            ):
                for cg in range(NC // 2):
                    if cg % 2 == 0:
                        xf = xp.tile([128, 4, D], F32, tag="xf")
                        nc.sync.dma_start(
                            xf[:],
                            x.ap().rearrange("(o p) d -> p o d", p=128)[
                                :, ds(cg * 2, 4), :])
                    tps = xtp.tile([128, 2, KD, 128], F32, tag="tx")
                    for j in range(2):
                        for kd in range(KD):
                            nc.tensor.transpose(
                                tps[:, j, kd, :],
                                xf[:, (cg % 2) * 2 + j, ds(kd * 128, 128)],
                                ident[:])
                    # ACT is idle through setup; keep copies off the DVE
                    # critical chain that gates the LayerNorm activations
                    nc.scalar.activation(
                        xT[:, :, ds(cg * 256, 256)].rearrange(
                            "p k (j c) -> p j k c", j=2), tps[:], AF.Copy)

            # ---- Q^T/K^T (head 0 -> SBUF, head 1 -> DRAM) + V/LN ----
            vsb = sp.tile([128, NC, HPC, 64], BF16)
            s1 = sp.tile([128, NC, HPC], F32)
            sqs = sp.tile([128, NC, HPC, 64], BF16)
            s2 = sp.tile([128, NC, HPC], F32)
            mu = sp.tile([128, NC, HPC, 1], F32)
            var = sp.tile([128, NC, HPC, 1], F32)
            rstd = sp.tile([128, NC, HPC, 1], F32)
            with (
                tc.tile_pool(name="qk_psum", bufs=2, space="PSUM") as qpp,
                tc.tile_pool(name="v_psum", bufs=2, space="PSUM") as vpp,
                tc.tile_pool(name="qk_st", bufs=2) as qst,
            ):
                def emit_qk(h):
                    for qi, w_s in ((0, wq_s), (1, wk_s)):
                        for nb in range(N // 1024):
                            ps = qpp.tile([64, 1024], F32, tag="psq")
                            for s in range(2):
                                for kd in range(KD):
                                    nc.tensor.matmul(
                                        ps[:, ds(s * 512, 512)],
                                        w_s[:, kd, ds(h * 64, 64)],
                                        xT[:, kd,
                                           ds(nb * 1024 + s * 512, 512)],
                                        start=(kd == 0), stop=(kd == KD - 1))
                            if h == 0:
                                dst = QT if qi == 0 else KT
                                nc.scalar.activation(
                                    dst[:, ds(nb * 1024, 1024)], ps[:],
                                    AF.Copy)
                            else:
                                stg = qst.tile([64, 1024], BF16, tag="stg")
                                nc.vector.tensor_copy(stg[:], ps[:])
                                nc.scalar.dma_start(
                                    qk_dr[qi, :, ds(nb * 1024, 1024)], stg[:])

                emit_qk(0)
                # V projection: 4 chunks per PSUM bank
                for cg in range(NC // 4):
                    vps = vpp.tile([128, 4, 128], F32, tag="vps")
                    for j in range(4):
                        for kd in range(KD):
                            nc.tensor.matmul(
                                vps[:, j, :],
                                xT[:, kd, ds((cg * 4 + j) * 128, 128)],
                                wv_s[:, kd, :],
                                start=(kd == 0), stop=(kd == KD - 1))
                    nc.vector.tensor_copy(
                        vsb[:, ds(cg * 4, 4), :, :],
                        vps[:].rearrange("p j (h d) -> p j h d", h=HPC))
                emit_qk(1)

            # ---- batched LayerNorm over dim_head ----
            nc.vector.tensor_reduce(
                s1.rearrange("p a b -> p (a b)"), vsb[:], axis=AX.X,
                op=ALU.add)
            nc.vector.tensor_tensor(sqs[:], vsb[:], vsb[:], ALU.mult)
            nc.vector.tensor_reduce(
                s2.rearrange("p a b -> p (a b)"), sqs[:], axis=AX.X,
                op=ALU.add)
            muf = mu.rearrange("p a b c -> p (a b c)")
            varf = var.rearrange("p a b c -> p (a b c)")
            s1f = s1.rearrange("p a b -> p (a b)")
            s2f = s2.rearrange("p a b -> p (a b)")
            nc.vector.tensor_scalar_mul(muf, s1f, 1.0 / 64.0)
            # var = s2/64 - mu^2   (as (s2*(1/64) - mu) ... need mu^2)
            nc.vector.scalar_tensor_tensor(
                varf, muf, -1.0, muf, ALU.mult, ALU.mult)   # -mu^2
            nc.vector.scalar_tensor_tensor(
                varf, s2f, 1.0 / 64.0, varf, ALU.mult, ALU.add)
            nc.vector.tensor_scalar_add(varf, varf, LN_EPS)
            # rstd = exp(-0.5 * ln(var + eps))
            nc.scalar.activation(varf, varf, AF.Ln)
            nc.scalar.activation(
                rstd.rearrange("p a b c -> p (a b c)"), varf,
                AF.Exp, scale=-0.5)
            # vln = (vsb - mu) * rstd * gamma + beta   (broadcast APs)
            mu_b = mu[:].broadcast_to([128, NC, HPC, 64])
            rstd_b = rstd[:].broadcast_to([128, NC, HPC, 64])
            gb = gbc[:].broadcast_to([128, NC, HPC, 64])
            bb = bbc[:].broadcast_to([128, NC, HPC, 64])
            nc.vector.tensor_tensor(vln[:], vsb[:], mu_b, ALU.subtract)
            nc.vector.tensor_tensor(vln[:], vln[:], rstd_b, ALU.mult)
            nc.vector.tensor_tensor(vln[:], vln[:], gb, ALU.mult)
            nc.vector.tensor_tensor(vln[:], vln[:], bb, ALU.add)
            nc.vector.tensor_copy(vones[:, :, :, 0:64], vln[:])

        # ================= main: per-head production + applications =========
        NSPILL = 20                 # head-1 P chunks spilled to DRAM
        with tc.tile_pool(name="pmain", bufs=1) as pm:
            P_sb = pm.tile([128, NC, N], FP8)
            P_dr = dram.tile([NSPILL, 128, N], FP8)

            def app_lhs(h, app, mp):
                return (vones[:, ds(2 * mp, 2), h, :] if app == 1
                        else tq[:, ds(2 * mp, 2), :])

            def emit_app_mms(h, app, apo, uTs, nbp):
                """acc matmuls + copy-out, in NB//nbp n-passes. Yields
                after every mp-group so prod-h1 can interleave."""
                for ip in range(NB // nbp):
                    acc = apo.tile([TW, nbp, 512], F32, tag="acc")
                    for mp in range(NC // 2):
                        lhs = app_lhs(h, app, mp)
                        for j in range(nbp):
                            nc.tensor.matmul(
                                acc[:, j, :], lhs,
                                P_sb[:, ds(2 * mp, 2),
                                     ds((ip * nbp + j) * 512, 512)],
                                start=(mp == 0), stop=(mp == NC // 2 - 1),
                                perf_mode=DR)
                        yield
                    nc.vector.tensor_copy(
                        uTs.rearrange("p (b n) -> p b n", n=512)[
                            :, ds(ip * nbp, nbp), :], acc[:])
                    yield

            def emit_app_tail(h, app, trp, uTs):
                """transpose back (8 chunks/bank) + batched epilogue."""
                for pk in range(NC // 8):
                    tp = trp.tile([128, 8, TW], BF16, tag="tp")
                    for j in range(8):
                        nc.tensor.transpose(
                            tp[:, j, :],
                            uTs[:, ds((pk * 8 + j) * 128, 128)],
                            ident_b[0:TW, 0:TW])
                    nc.vector.tensor_copy(uSt[:, ds(pk * 8, 8), :], tp[:])
                    yield
                if app == 1:
                    nc.vector.reciprocal(rinv[:], uSt[:, :, 64:65])
                rb = rinv[:].broadcast_to([128, NC, 64])
                # tq = (uSt * scale) * (1/r)  [fp8, feeds next app]
                nc.vector.scalar_tensor_tensor(
                    tq[:, :, 0:64], uSt[:, :, 0:64],
                    T_SCALE if app == 1 else 1.0, rb, ALU.mult, ALU.mult)
                yh = yt[:, :, h, :]
                if app == 1:
                    nc.vector.tensor_scalar_mul(
                        yh, vln[:, :, h, :], cbc[:, ds(h * 4, 1)])
                # y += (c_k/16) * tq
                nc.vector.scalar_tensor_tensor(
                    yh, tq[:, :, 0:64], cbc16[:, ds(h * 4 + app, 1)],
                    yh, ALU.mult, ALU.add)
                yield

            # ---- head 0 production: full SBUF residency, 8-bank PSUM ----
            with tc.tile_pool(name="prod_psum", bufs=2, space="PSUM") as ppp:
                for mc in range(NC):
                    for half in range(2):
                        ps = ppp.tile([128, 2048], F32, tag="s")
                        for q in range(4):
                            nc.tensor.matmul(
                                ps[:, ds(q * 512, 512)],
                                KT[:, ds(mc * 128, 128)],
                                QT[:, ds(half * 2048 + q * 512, 512)],
                                start=True, stop=True)
                        nc.scalar.activation(
                            P_sb[:, mc, ds(half * 2048, 2048)], ps[:],
                            AF.Exp, scale=SM_SCALE)

            # head-1 Q^T/K^T reload (waits on prod-h0's last reads)
            nc.sync.dma_start(QT[:], qk_dr[0])
            nc.sync.dma_start(KT[:], qk_dr[1])

            # ---- overlap: apps h0 (PE/DVE) || production h1 (ACT) ----
            # PSUM: prod 2x[128,1024]=4 banks, acc [80,2,512]=2, tr 2.
            with (
                tc.tile_pool(name="ov_prod", bufs=2, space="PSUM") as ovp,
                tc.tile_pool(name="ov_acc", bufs=1, space="PSUM") as apo2,
                tc.tile_pool(name="ov_tr", bufs=2, space="PSUM") as trp2,
                tc.tile_pool(name="ov_sb", bufs=1) as ovs,
                tc.tile_pool(name="spill_sb", bufs=2) as sps,
            ):
                uTs0 = ovs.tile([TW, N], BF16)

                def prod1_unit(mc, qtr, stg):
                    """one quarter-chunk: 2 S-matmuls + exp."""
                    ps = ovp.tile([128, 1024], F32, tag="s1")
                    for q in range(2):
                        nc.tensor.matmul(
                            ps[:, ds(q * 512, 512)],
                            KT[:, ds(mc * 128, 128)],
                            QT[:, ds(qtr * 1024 + q * 512, 512)],
                            start=True, stop=True)
                    dst = (stg[:, ds(qtr * 1024, 1024)] if stg is not None
                           else P_sb[:, mc, ds(qtr * 1024, 1024)])
                    nc.scalar.activation(dst, ps[:], AF.Exp, scale=SM_SCALE)

                def prod1_spill_units():
                    for mc in range(NSPILL):
                        stg = sps.tile([128, N], FP8, tag="pt")
                        for qtr in range(4):
                            prod1_unit(mc, qtr, stg)
                            yield
                        nc.scalar.dma_start(P_dr[mc], stg[:])

                def apps0_units():
                    for app in range(1, 4):
                        yield from emit_app_mms(0, app, apo2, uTs0, nbp=2)
                        yield from emit_app_tail(0, app, trp2, uTs0)

                gp = prod1_spill_units()
                ga = apps0_units()
                done_p = done_a = False
                while not (done_p and done_a):
                    if not done_p:
                        done_p = next(gp, "end") == "end"
                    if not done_a:
                        for _ in range(3):
                            if next(ga, "end") == "end":
                                done_a = True
                                break
                # reload spilled chunks into P_sb (slots free after apps
                # h0; DMAs run during prod-h1's direct tail)
                for p in range(NSPILL // 2):
                    nc.sync.dma_start(
                        P_sb[:, ds(2 * p, 2), :],
                        P_dr[ds(2 * p, 2)].rearrange("c p n -> p c n"))
                # direct-to-SBUF tail of prod h1 (slots freed by apps h0)
                for mc in range(NSPILL, NC):
                    for qtr in range(4):
                        prod1_unit(mc, qtr, None)
                # app1 of head 1: 2-bank accumulator so its early pairs can
                # run under prod-h1's trailing exps
                for _ in emit_app_mms(1, 1, apo2, uTs0, nbp=2):
                    pass
                for _ in emit_app_tail(1, 1, trp2, uTs0):
                    pass

            # ---- apps 2,3 of head 1 (full 8-bank accumulator) ----
            for app in range(2, 4):
                with tc.tile_pool(name="uts", bufs=1) as up:
                    uTs = up.tile([TW, N], BF16)
                    with tc.tile_pool(name="acc_psum", bufs=1,
                                      space="PSUM") as apo:
                        for _ in emit_app_mms(1, app, apo, uTs, nbp=NB):
                            pass
                    with tc.tile_pool(name="tr_psum", bufs=2,
                                      space="PSUM") as trp:
                        for _ in emit_app_tail(1, app, trp, uTs):
                            pass

            # ---- output projection (both heads fused in PSUM) ----
            with (
                tc.tile_pool(name="ty_psum", bufs=2, space="PSUM") as typ,
                tc.tile_pool(name="o_psum", bufs=2, space="PSUM") as opp,
                tc.tile_pool(name="o_st", bufs=3) as ost,
            ):
                for ci in range(NC):
                    ty = typ.tile([64, HPC, 128], F32, tag="ty")
                    for h in range(HPC):
                        nc.tensor.transpose(
                            ty[:, h, :], yt[:, ci, h, :], ident[:])
                    yst = ost.tile([64, HPC, 128], BF16, tag="yst")
                    nc.scalar.activation(yst[:], ty[:], AF.Copy)
                    po = opp.tile([128, 256], F32, tag="po")
                    for h in range(HPC):
                        nc.tensor.matmul(
                            po[:], yst[:, h, :], wo_s[:, h, :],
                            start=(h == 0), stop=(h == HPC - 1))
                    ob = ost.tile([128, 256], BF16, tag="ob")
                    nc.vector.tensor_copy(ob[:], po[:])
                    nc.sync.dma_start(
                        bounce_in.rearrange("(c p) d -> p c d", p=128)[
                            :, ci, :], ob[:])

        # ---------- ReduceScatter (bf16) over the batch group + output ------
        if not collective:
            nc.sync.dma_start(
                out_d.ap().rearrange("(c p) d -> p c d", p=128),
                bounce_in.rearrange("(c p) d -> p c d", p=128)[
                    :, 0:NC // 4, :])
            return
        G = len(replica_groups[0])
        bounce_out = dram.tile([N // G, D], BF16)
        nc.gpsimd.collective_compute(
            "ReduceScatter", ALU.add, replica_groups=replica_groups,
            ins=[bounce_in.opt()], outs=[bounce_out.opt()])
        nc.sync.dma_start(out_d.ap(), bounce_out[:])


# ----------------------------------------------------------------------------
# host-side entry point
# ----------------------------------------------------------------------------

_CACHED = {}


def _get_compiled(N=4096, n_cores=8, p_dtype=FP8):
    key = (N, n_cores, p_dtype)
    if key not in _CACHED:
        groups = [list(range(g * 4, g * 4 + 4)) for g in range(2)] \
            if n_cores == 8 else [list(range(n_cores))]
        nc = bacc.Bacc("TRN2", target_bir_lowering=False, debug=False,
                       num_devices=n_cores)
        build_kernel(nc, N=N, replica_groups=groups, p_dtype=p_dtype)
        nc.compile()
        _CACHED[key] = nc
    return _CACHED[key]


def make_in_maps(x, Wq, Wk, Wv, Wo, gamma, beta, coeffs, n_cores=8):
    """Shard full inputs into per-core input maps (batch + head-pair)."""
    x = np.asarray(x, np.float32)
    Wq = np.asarray(Wq, np.float32)
    Wk = np.asarray(Wk, np.float32)
    Wv = np.asarray(Wv, np.float32)
    Wo = np.asarray(Wo, np.float32)
    gamma = np.asarray(gamma, np.float32)
    beta = np.asarray(beta, np.float32)
    coeffs = np.asarray(coeffs, np.float32)
    g2 = np.concatenate([gamma, gamma]).reshape(1, 128).copy()
    b2 = np.concatenate([beta, beta]).reshape(1, 128).copy()
    in_maps = []
    for core in range(n_cores):
        b = core // 4 if n_cores == 8 else 0
        hp = core % 4 if n_cores == 8 else core
        cols = slice(hp * 128, (hp + 1) * 128)
        in_maps.append({
            "x": np.ascontiguousarray(x[b]),
            "wq": np.ascontiguousarray(Wq[:, cols]),
            "wk": np.ascontiguousarray(Wk[:, cols]),
            "wv": np.ascontiguousarray(Wv[:, cols]),
            "wo": np.ascontiguousarray(Wo[cols, :]),
            "gamma2": g2,
            "beta2": b2,
            "cf": np.ascontiguousarray(coeffs[2 * hp: 2 * hp + 2].reshape(1, 8)),
        })
    return in_maps


def kernel(x, Wq, Wk, Wv, Wo, gamma, beta, coeffs, trace=False):
    from concourse.bass_utils import run_bass_kernel_spmd

    n_cores = 8
    nc = _get_compiled(4096, n_cores)
    in_maps = make_in_maps(x, Wq, Wk, Wv, Wo, gamma, beta, coeffs, n_cores)
    res = run_bass_kernel_spmd(nc, in_maps, core_ids=list(range(n_cores)),
                               trace=trace)
    # each core returns its ReduceScatter shard: rank k of a batch group
    # holds rows [k*N/4, (k+1)*N/4) of that batch's output
    N = 4096
    out = np.empty((2, N, 256), np.float32)
    for b, cores in enumerate([[0, 1, 2, 3], [4, 5, 6, 7]]):
        for k, c in enumerate(cores):
            shard = np.asarray(res.results[c]["out"]).astype(np.float32)
            out[b, k * (N // 4):(k + 1) * (N // 4)] = shard
    if trace:
        kernel.last_result = res
    return out



# revision 9
# speedup vs baseline: 1.3248x; 1.3248x over previous
"""AGF attention (graph-filter attention) distributed Bass kernel for 8 TRN2
cores.

Sharding: batch x head-pair (data + head parallel). Core i handles batch
b = i//4 and heads {2*(i%4), 2*(i%4)+1}. Each core computes its partial
output projection (summed over its 2 heads); a bf16 ReduceScatter over the
4 cores of each batch produces row shards of that batch's [N, D] output,
which the host concatenates.

v3 design (per core), engine-balanced around the exp() bottleneck:
  - S^T is produced by fp8 DoubleRow matmuls (Q^T/K^T quantized to fp8 in a
    zero-padded [64, 2, N] pair layout, so the PSUM->SBUF projection copies
    double as the quantization) into a small PSUM ring.
  - P = exp(S^T/8) is split between ACT (native Exp -> fp8) and DVE (a
    Schraudolph-style exp: i8 = rint(log2e/8-scaled logit + bias) written
    through an int8 bitcast of the fp8 P tile; the int8 IS the fp8 bit
    pattern). This roughly doubles exp throughput, the kernel bottleneck.
  - P stays fully SBUF-resident per head ([128, NC, N] fp8); head 1 reuses
    head 0's chunk slots (stagger via a spare-slot tile) with no DRAM spill.
  - Graph-filter applications keep P as the *stationary* operand:
    u[q, d] = sum_k P^T[k, q] t[k, d] comes out in natural layout (no
    transpose-back), out-free = 64/65 cols, fp8 DR. app1 carries a ones
    column to produce the softmax row-sums.
  - Epilogues (t requant + y accumulate) are batched DVE ops; LayerNorm
    elementwise chain and misc copies run on the otherwise-idle GPSIMD.
  - x^T is produced by xbar DMA transposes (DRAM bf16 bounce), not PE.
"""

import numpy as np

import concourse.mybir as mybir
import concourse.tile as tile
from concourse import bacc
from concourse.bass import ds
from concourse.masks import make_identity

dt = mybir.dt
F32 = dt.float32
BF16 = dt.bfloat16
FP8 = dt.float8e4
I8 = dt.int8
AF = mybir.ActivationFunctionType
ALU = mybir.AluOpType
AX = mybir.AxisListType
DR = mybir.MatmulPerfMode.DoubleRow

D = 256      # model dim
DH = 64      # head dim
HPC = 2      # heads per core
KD = D // 128
LN_EPS = 1e-5
SM_SCALE = 0.125  # 1/sqrt(DH)
T_SCALE = 16.0

# Schraudolph-style exp constants (fp8e4m3 bit domain, rint semantics):
#   i8 = rint(A8C * s + B8C);  bitcast_e4m3(i8) ~= exp(s * SM_SCALE)
A8C = 8.0 / np.log(2.0) * SM_SCALE
B8C = 55.64

STAG = 8          # head-1 chunk-slot stagger (spare P slots)
DVE_FRAC = 0.40   # fraction of exp slots on DVE
ACT_WARM = 10     # first slots forced to ACT (DVE busy with setup)


def _dve_slot(i, n_total):
    if i < ACT_WARM:
        return False
    return int((i + 1) * DVE_FRAC) > int(i * DVE_FRAC)


def build_kernel(nc, N=4096, replica_groups=((0, 1, 2, 3), (4, 5, 6, 7)),
                 p_dtype=FP8, collective=True):
    NC = N // 128
    replica_groups = [list(g) for g in replica_groups]
    G = len(replica_groups[0]) if collective else 4

    x = nc.dram_tensor("x", [N, D], F32, kind="ExternalInput")
    wq_d = nc.dram_tensor("wq", [D, HPC * DH], F32, kind="ExternalInput")
    wk_d = nc.dram_tensor("wk", [D, HPC * DH], F32, kind="ExternalInput")
    wv_d = nc.dram_tensor("wv", [D, HPC * DH], F32, kind="ExternalInput")
    wo_d = nc.dram_tensor("wo", [HPC * DH, D], F32, kind="ExternalInput")
    gamma_d = nc.dram_tensor("gamma2", [1, HPC * DH], F32, kind="ExternalInput")
    beta_d = nc.dram_tensor("beta2", [1, HPC * DH], F32, kind="ExternalInput")
    cf_d = nc.dram_tensor("cf", [1, HPC * 4], F32, kind="ExternalInput")
    out_d = nc.dram_tensor("out", [N // G, D], BF16, kind="ExternalOutput")

    with tile.TileContext(nc) as tc:
        _body(nc, tc, N, NC, replica_groups,
              x, wq_d, wk_d, wv_d, wo_d, gamma_d, beta_d, cf_d, out_d,
              collective)
    return nc


def _body(nc, tc, N, NC, replica_groups,
          x, wq_d, wk_d, wv_d, wo_d, gamma_d, beta_d, cf_d, out_d,
          collective=True):
    HN = NC * N            # flat columns per head (chunk-major)

    with (
        tc.tile_pool(name="persist", bufs=1) as pp,
        tc.tile_pool(name="dram", bufs=1, space="DRAM") as dram,
    ):
        # ---------- persistent tiles ----------
        ident_b = pp.tile([128, 128], BF16)

        cbc = pp.tile([128, 8], F32)       # coeffs broadcast [h*4 + k]
        cbc16 = pp.tile([128, 8], F32)     # coeffs / T_SCALE

        P = pp.tile([128, NC, N], FP8)     # current head's P^T (chunk-major)
        QT8 = pp.tile([64, 2, N], FP8)     # zero-padded pair layout
        KT8 = pp.tile([64, 2, N], FP8)
        vln = pp.tile([128, NC, HPC, 64], BF16)
        vln8 = pp.tile([128, NC, HPC, 80], FP8)   # [v_ln | 1 | 0-pad]
        t8 = pp.tile([128, NC, 64], FP8)
        yt = pp.tile([128, NC, HPC, 64], BF16)
        rinv = pp.tile([128, NC, 1], F32)
        wo_s = pp.tile([64, HPC, 256], BF16)
        gbc = pp.tile([128, 1, HPC, 64], F32)
        bbc = pp.tile([128, 1, HPC, 64], F32)

        qk8_dr = dram.tile([2, 64, N], FP8)       # head-1 Q^T/K^T (real half)
        xb_dr = dram.tile([N, D], BF16)
        bounce_in = dram.tile([N, D], BF16)

        nc.vector.memset(QT8[:, 1, :], 0.0)
        nc.vector.memset(KT8[:, 1, :], 0.0)
        nc.vector.memset(vln8[:, :, :, 64:65], 1.0)
        nc.vector.memset(vln8[:, :, :, 65:80], 0.0)
        make_identity(nc, ident_b)

        # ================= setup =================
        with (
            tc.tile_pool(name="setup", bufs=1) as sp,
            tc.tile_pool(name="xio", bufs=2) as xp,
            tc.tile_pool(name="qk_psum", bufs=2, space="PSUM") as qpp,
            tc.tile_pool(name="v_psum", bufs=2, space="PSUM") as vpp,
            tc.tile_pool(name="qk_st", bufs=2) as qst,
            tc.tile_pool(name="vln_st", bufs=2) as vst,
        ):
            # constants
            c1 = sp.tile([1, 8], F32)
            nc.sync.dma_start(c1[:], cf_d.ap())
            nc.gpsimd.partition_broadcast(cbc[:], c1[:])
            nc.vector.tensor_scalar_mul(cbc16[:], cbc[:], 1.0 / T_SCALE)
            g1 = sp.tile([1, 128], F32)
            nc.sync.dma_start(g1[:], gamma_d.ap())
            nc.gpsimd.partition_broadcast(
                gbc[:].rearrange("p o a b -> p (o a b)"), g1[:])
            b1 = sp.tile([1, 128], F32)
            nc.sync.dma_start(b1[:], beta_d.ap())
            nc.gpsimd.partition_broadcast(
                bbc[:].rearrange("p o a b -> p (o a b)"), b1[:])

            # weights -> SBUF bf16
            wst = sp.tile([128, 3, KD, 128], F32)
            nc.sync.dma_start(
                wst[:, 0], wq_d.ap().rearrange("(o p) m -> p o m", p=128))
            nc.sync.dma_start(
                wst[:, 1], wk_d.ap().rearrange("(o p) m -> p o m", p=128))
            nc.sync.dma_start(
                wst[:, 2], wv_d.ap().rearrange("(o p) m -> p o m", p=128))
            wsb = sp.tile([128, 3, KD, 128], BF16)
            nc.vector.tensor_copy(wsb[:], wst[:])
            wo_f = sp.tile([64, HPC, 256], F32)
            nc.scalar.dma_start(
                wo_f[:], wo_d.ap().rearrange("(h d) m -> d h m", h=HPC))
            nc.vector.tensor_copy(wo_s[:], wo_f[:])

            # ---- per-half x -> x^T (bf16, DRAM bounce + xbar DMA transpose),
            # ---- then Q/K projections + V/LN for that half ----
            HW = N // 2
            xT = sp.tile([128, KD, HW], BF16)   # current half's x^T
            x_r = x.ap().rearrange("(o p) d -> p o d", p=128)
            xb_r = xb_dr[:].rearrange("(o p) d -> p o d", p=128)

            mu = sp.tile([128, 4, HPC, 1], F32)
            var = sp.tile([128, 4, HPC, 1], F32)
            rstd = sp.tile([128, 4, HPC, 1], F32)
            s1 = sp.tile([128, 4, HPC], F32)
            s2 = sp.tile([128, 4, HPC], F32)

            def emit_x_half(hf):
                for g in range(4):
                    gq = hf * 4 + g
                    xf = xp.tile([128, 4, D], F32, tag="xf")
                    nc.sync.dma_start(xf[:], x_r[:, ds(gq * 4, 4), :])
                    xb = xp.tile([128, 4, D], BF16, tag="xb")
                    nc.gpsimd.tensor_copy(xb[:], xf[:])
                    nc.scalar.dma_start(xb_r[:, ds(gq * 4, 4), :], xb[:])
                nc.sync.dma_start_transpose(
                    xT[:], xb_dr[ds(hf * HW, HW), :])

            def emit_qk(h, hf):
                """Q/K projection blocks for n-half hf of head h."""
                for qi in range(2):
                    for nb in range(4):
                        gb = hf * 4 + nb
                        ps = qpp.tile([64, 512], F32, tag="psq")
                        for kd in range(KD):
                            nc.tensor.matmul(
                                ps[:],
                                wsb[:, qi, kd, ds(h * 64, 64)],
                                xT[:, kd, ds(nb * 512, 512)],
                                start=(kd == 0), stop=(kd == KD - 1))
                        if h == 0:
                            dst = QT8 if qi == 0 else KT8
                            nc.scalar.activation(
                                dst[:, 0, ds(gb * 512, 512)], ps[:], AF.Copy)
                        else:
                            stg = qst.tile([64, 512], FP8, tag="stg")
                            nc.scalar.activation(stg[:], ps[:], AF.Copy)
                            nc.scalar.dma_start(
                                qk8_dr[qi, :, ds(gb * 512, 512)], stg[:])

            def emit_v_ln(hf, lq):
                """V projection + LayerNorm for local chunk-group lq."""
                q = hf * 4 + lq
                vps = vpp.tile([128, 4, 128], F32, tag="vps")
                for j in range(4):
                    for kd in range(KD):
                        nc.tensor.matmul(
                            vps[:, j, :],
                            xT[:, kd, ds((lq * 4 + j) * 128, 128)],
                            wsb[:, 2, kd, :],
                            start=(kd == 0), stop=(kd == KD - 1))
                vsb = vst.tile([128, 4, HPC, 64], BF16, tag="vsb")
                nc.vector.tensor_copy(
                    vsb[:], vps[:].rearrange("p j (h d) -> p j h d", h=HPC))
                # stats
                nc.vector.tensor_reduce(
                    s1[:].rearrange("p a b -> p (a b)"), vsb[:], axis=AX.X,
                    op=ALU.add)
                sqs = vst.tile([128, 4, HPC, 64], BF16, tag="sqs")
                nc.gpsimd.tensor_tensor(sqs[:], vsb[:], vsb[:], ALU.mult)
                nc.vector.tensor_reduce(
                    s2[:].rearrange("p a b -> p (a b)"), sqs[:], axis=AX.X,
                    op=ALU.add)
                muf = mu[:].rearrange("p a b c -> p (a b c)")
                varf = var[:].rearrange("p a b c -> p (a b c)")
                s1f = s1[:].rearrange("p a b -> p (a b)")
                s2f = s2[:].rearrange("p a b -> p (a b)")
                nc.vector.tensor_scalar_mul(muf, s1f, 1.0 / 64.0)
                nc.vector.scalar_tensor_tensor(
                    varf, muf, -1.0, muf, ALU.mult, ALU.mult)   # -mu^2
                nc.vector.scalar_tensor_tensor(
                    varf, s2f, 1.0 / 64.0, varf, ALU.mult, ALU.add)
                nc.vector.tensor_scalar_add(varf, varf, LN_EPS)
                nc.scalar.activation(varf, varf, AF.Ln)
                nc.scalar.activation(
                    rstd[:].rearrange("p a b c -> p (a b c)"), varf,
                    AF.Exp, scale=-0.5)
                # vln = (vsb - mu) * rstd * gamma + beta  (Pool chain)
                sh = [128, 4, HPC, 64]
                vt = vst.tile(sh, F32, tag="vt")
                nc.gpsimd.tensor_tensor(
                    vt[:], vsb[:], mu[:].broadcast_to(sh), ALU.subtract)
                nc.gpsimd.tensor_tensor(
                    vt[:], vt[:], rstd[:].broadcast_to(sh), ALU.mult)
                nc.gpsimd.tensor_tensor(
                    vt[:], vt[:], gbc[:].broadcast_to(sh), ALU.mult)
                vsl = vln[:, ds(q * 4, 4), :, :]
                nc.gpsimd.tensor_tensor(
                    vsl, vt[:], bbc[:].broadcast_to(sh), ALU.add)
                nc.gpsimd.tensor_copy(vln8[:, ds(q * 4, 4), :, 0:64], vsl)

            for hf in range(2):
                emit_x_half(hf)
                emit_qk(0, hf)
                for lq in range(4):
                    emit_v_ln(hf, lq)
                emit_qk(1, hf)

            # y init: y_h = c0_h * vln_h
            for h in range(HPC):
                nc.vector.tensor_scalar_mul(
                    yt[:, :, h, :], vln[:, :, h, :], cbc[:, ds(h * 4, 1)])

        # ================= main: exp slots + applications =================
        n_slots_h0 = HN // 2048
        n_slots_h1 = (HN + 1535) // 1536
        total_slots = n_slots_h0 + n_slots_h1
        Pflat = P[:].rearrange("p c n -> p (c n)")

        def consume(slot_ap, pieces, gslot):
            off = 0
            for dst_ap, w in pieces:
                src = slot_ap[:, ds(off, w)]
                if _dve_slot(gslot, total_slots):
                    nc.vector.tensor_scalar(
                        dst_ap.bitcast(I8), src, A8C, B8C, ALU.mult, ALU.add)
                else:
                    nc.scalar.activation(dst_ap, src, AF.Exp, scale=SM_SCALE)
                off += w

        def prod_gen(h, ring, slot_w, gslot0, pw):
            """Produce + exp head h's P in `slot_w`-column PSUM slots.
            pw(flat_col, width) -> [(dst AP, width), ...] in P storage."""
            col = 0
            s = 0
            while col < HN:
                w = min(slot_w, HN - col)
                sl = ring.tile([128, slot_w], F32, tag="sl")
                for q in range(w // 512):
                    fc = col + q * 512
                    mc, qc = fc // N, fc % N
                    nc.tensor.matmul(
                        sl[:, ds(q * 512, 512)],
                        KT8[:, :, ds(mc * 128, 128)],
                        QT8[:, :, ds(qc, 512)],
                        start=True, stop=True, perf_mode=DR)
                consume(sl, pw(col, w), gslot0 + s)
                col += w
                s += 1
                yield

        # head-1 chunk c -> spare slot c (c < STAG) else P slot c - STAG
        def app_lhsT(h, kp, j, P_spare):
            """Stationary P pair (chunks 2kp, 2kp+1) for q-window j."""
            if h == 0:
                return P[:, ds(2 * kp, 2), ds(j * 128, 128)]
            c = 2 * kp
            if c + 2 <= STAG:
                return P_spare[:, ds(c, 2), ds(j * 128, 128)]
            return P[:, ds(c - STAG, 2), ds(j * 128, 128)]

        def h1_pw(P_spare):
            Psf = P_spare[:].rearrange("p c n -> p (c n)")
            sb = STAG * N

            def pw(col, w):
                # [col, col+w) relative to head start; spare covers first sb
                if col + w <= sb:
                    return [(Psf[:, ds(col, w)], w)]
                if col >= sb:
                    return [(Pflat[:, ds(col - sb, w)], w)]
                w0 = sb - col
                return [(Psf[:, ds(col, w0)], w0),
                        (Pflat[:, ds(0, w - w0)], w - w0)]
            return pw

        def apps_gen(h, accp, P_spare):
            """Graph-filter applications for head h (natural-layout form)."""
            for app in (1, 2, 3):
                if app == 1:
                    groups = [list(range(g * 7, min(NC, g * 7 + 7)))
                              for g in range(5)]
                    W = 65
                else:
                    groups = [list(range(g * 8, g * 8 + 8)) for g in range(4)]
                    W = 64
                for cl in groups:
                    L = len(cl)
                    acc = accp.tile([128, 512], F32, tag="acc")
                    for ji, j in enumerate(cl):
                        for kp in range(NC // 2):
                            rhs = (vln8[:, ds(2 * kp, 2), h, 0:W] if app == 1
                                   else t8[:, ds(2 * kp, 2), :])
                            nc.tensor.matmul(
                                acc[:, ds(ji * W, W)],
                                app_lhsT(h, kp, j, P_spare), rhs,
                                start=(kp == 0), stop=(kp == NC // 2 - 1),
                                perf_mode=DR)
                        yield
                    accg = acc[:, 0:L * W].rearrange("p (a b) -> p a b", b=W)
                    c0 = cl[0]
                    if app == 1:
                        nc.vector.reciprocal(
                            rinv[:, ds(c0, L), :], accg[:, :, 64:65])
                    # t8 = (u * scale) * (1/r)
                    nc.vector.scalar_tensor_tensor(
                        t8[:, ds(c0, L), :], accg[:, :, 0:64],
                        T_SCALE if app == 1 else 1.0,
                        rinv[:, ds(c0, L), :].broadcast_to([128, L, 64]),
                        ALU.mult, ALU.mult)
                    # y += (c_app / T_SCALE) * t8
                    nc.vector.scalar_tensor_tensor(
                        yt[:, ds(c0, L), h, :], t8[:, ds(c0, L), :],
                        cbc16[:, ds(h * 4 + app, 1)],
                        yt[:, ds(c0, L), h, :], ALU.mult, ALU.add)
                    yield

        def outproj_gen(opp, ost):
            for cg in range(NC // 4):
                ob = ost.tile([128, 4, 256], BF16, tag="ob")
                for j in range(4):
                    ci = cg * 4 + j
                    ty = opp.tile([64, HPC, 128], BF16, tag="ty")
                    for h in range(HPC):
                        nc.tensor.transpose(
                            ty[:, h, :], yt[:, ci, h, :], ident_b[:])
                    yst = ost.tile([64, HPC, 128], BF16, tag="yst")
                    nc.vector.tensor_copy(yst[:], ty[:])
                    po = opp.tile([128, 256], F32, tag="po")
                    for h in range(HPC):
                        nc.tensor.matmul(
                            po[:], yst[:, h, :], wo_s[:, h, :],
                            start=(h == 0), stop=(h == HPC - 1))
                    nc.scalar.activation(ob[:, j, :], po[:], AF.Copy)
                    yield
                nc.sync.dma_start(
                    bounce_in[:].rearrange("(c p) d -> p c d", p=128)[
                        :, ds(cg * 4, 4), :],
                    ob[:])
                yield

        with tc.tile_pool(name="spare", bufs=1) as spare_pool:
            P_spare = spare_pool.tile([128, STAG, N], FP8)

            # ---- head 0 slots (2048-wide, 8-bank ring) ----
            with tc.tile_pool(name="ring0", bufs=2, space="PSUM") as ring0:
                g0 = prod_gen(0, ring0, 2048, 0,
                              lambda c, w: [(Pflat[:, ds(c, w)], w)])
                for _ in g0:
                    pass

            # head-1 Q^T/K^T reload (zero halves already in place)
            nc.sync.dma_start(QT8[:, 0, :], qk8_dr[0])
            nc.sync.dma_start(KT8[:, 0, :], qk8_dr[1])

            # ---- head 1 slots (1536-wide ring) + head-0 apps interleaved ----
            with (
                tc.tile_pool(name="ring1", bufs=2, space="PSUM") as ring1,
                tc.tile_pool(name="acc0", bufs=1, space="PSUM") as acc0,
            ):
                g1 = prod_gen(1, ring1, 1536, n_slots_h0, h1_pw(P_spare))
                ga = apps_gen(0, acc0, P_spare)
                done_p = done_a = False
                while not (done_p and done_a):
                    if not done_p:
                        done_p = next(g1, "end") == "end"
                    if not done_a:
                        for _ in range(8):
                            if next(ga, "end") == "end":
                                done_a = True
                                break

            # ---- tail: head-1 apps + output projection ----
            with (
                tc.tile_pool(name="acc1", bufs=2, space="PSUM") as acc1,
                tc.tile_pool(name="op_psum", bufs=2, space="PSUM") as opp,
                tc.tile_pool(name="o_st", bufs=2) as ost,
            ):
                ga1 = apps_gen(1, acc1, P_spare)
                for _ in ga1:
                    pass
                for _ in outproj_gen(opp, ost):
                    pass

        # ---------- ReduceScatter (bf16) over the batch group + output ------
        if not collective:
            nc.sync.dma_start(
                out_d.ap().rearrange("(c p) d -> p c d", p=128),
                bounce_in[:].rearrange("(c p) d -> p c d", p=128)[
                    :, 0:NC // 4, :])
            return
        G = len(replica_groups[0])
        bounce_out = dram.tile([N // G, D], BF16)
        nc.gpsimd.collective_compute(
            "ReduceScatter", ALU.add, replica_groups=replica_groups,
            ins=[bounce_in.opt()], outs=[bounce_out.opt()])
        nc.sync.dma_start(out_d.ap(), bounce_out[:])


# ----------------------------------------------------------------------------
# host-side entry point
# ----------------------------------------------------------------------------

_CACHED = {}


def _get_compiled(N=4096, n_cores=8, p_dtype=FP8):
    key = (N, n_cores, p_dtype)
    if key not in _CACHED:
        groups = [list(range(g * 4, g * 4 + 4)) for g in range(2)] \
            if n_cores == 8 else [list(range(n_cores))]
        nc = bacc.Bacc("TRN2", target_bir_lowering=False, debug=False,
                       num_devices=n_cores)
        build_kernel(nc, N=N, replica_groups=groups, p_dtype=p_dtype)
        nc.compile()
        _CACHED[key] = nc
    return _CACHED[key]


def make_in_maps(x, Wq, Wk, Wv, Wo, gamma, beta, coeffs, n_cores=8):
    """Shard full inputs into per-core input maps (batch + head-pair)."""
    x = np.asarray(x, np.float32)
    Wq = np.asarray(Wq, np.float32)
    Wk = np.asarray(Wk, np.float32)
    Wv = np.asarray(Wv, np.float32)
    Wo = np.asarray(Wo, np.float32)
    gamma = np.asarray(gamma, np.float32)
    beta = np.asarray(beta, np.float32)
    coeffs = np.asarray(coeffs, np.float32)
    g2 = np.concatenate([gamma, gamma]).reshape(1, 128).copy()
    b2 = np.concatenate([beta, beta]).reshape(1, 128).copy()
    in_maps = []
    for core in range(n_cores):
        b = core // 4 if n_cores == 8 else 0
        hp = core % 4 if n_cores == 8 else core
        cols = slice(hp * 128, (hp + 1) * 128)
        in_maps.append({
            "x": np.ascontiguousarray(x[b]),
            "wq": np.ascontiguousarray(Wq[:, cols]),
            "wk": np.ascontiguousarray(Wk[:, cols]),
            "wv": np.ascontiguousarray(Wv[:, cols]),
            "wo": np.ascontiguousarray(Wo[cols, :]),
            "gamma2": g2,
            "beta2": b2,
            "cf": np.ascontiguousarray(coeffs[2 * hp: 2 * hp + 2].reshape(1, 8)),
        })
    return in_maps


def kernel(x, Wq, Wk, Wv, Wo, gamma, beta, coeffs, trace=False):
    from concourse.bass_utils import run_bass_kernel_spmd

    n_cores = 8
    nc = _get_compiled(4096, n_cores)
    in_maps = make_in_maps(x, Wq, Wk, Wv, Wo, gamma, beta, coeffs, n_cores)
    res = run_bass_kernel_spmd(nc, in_maps, core_ids=list(range(n_cores)),
                               trace=trace)
    # each core returns its ReduceScatter shard: rank k of a batch group
    # holds rows [k*N/4, (k+1)*N/4) of that batch's output
    N = 4096
    out = np.empty((2, N, 256), np.float32)
    for b, cores in enumerate([[0, 1, 2, 3], [4, 5, 6, 7]]):
        for k, c in enumerate(cores):
            shard = np.asarray(res.results[c]["out"]).astype(np.float32)
            out[b, k * (N // 4):(k + 1) * (N // 4)] = shard
    if trace:
        kernel.last_result = res
    return out


# revision 54
# speedup vs baseline: 1.6480x; 1.2440x over previous
"""AGF attention (graph-filter attention) distributed Bass kernel for 8 TRN2
cores.

Sharding: batch x head-pair (data + head parallel). Core i handles batch
b = i//4 and heads {2*(i%4), 2*(i%4)+1}. Each core computes its partial
output projection (summed over its 2 heads); a bf16 ReduceScatter over the
4 cores of each batch produces row shards of that batch's [N, D] output,
which the host concatenates.

v3 design (per core), engine-balanced around the exp() bottleneck:
  - S^T is produced by fp8 DoubleRow matmuls (Q^T/K^T quantized to fp8 in a
    zero-padded [64, 2, N] pair layout, so the PSUM->SBUF projection copies
    double as the quantization) into a small PSUM ring.
  - P = exp(S^T/8) is split between ACT (native Exp -> fp8) and DVE (a
    Schraudolph-style exp: i8 = rint(log2e/8-scaled logit + bias) written
    through an int8 bitcast of the fp8 P tile; the int8 IS the fp8 bit
    pattern). This roughly doubles exp throughput, the kernel bottleneck.
  - P stays fully SBUF-resident per head ([128, NC, N] fp8); head 1 reuses
    head 0's chunk slots (stagger via a spare-slot tile) with no DRAM spill.
  - Graph-filter applications keep P as the *stationary* operand:
    u[q, d] = sum_k P^T[k, q] t[k, d] comes out in natural layout (no
    transpose-back), out-free = 64/65 cols, fp8 DR. app1 carries a ones
    column to produce the softmax row-sums.
  - Epilogues (t requant + y accumulate) are batched DVE ops; LayerNorm
    elementwise chain and misc copies run on the otherwise-idle GPSIMD.
  - x^T is produced by xbar DMA transposes (DRAM bf16 bounce), not PE.
"""

import numpy as np

import concourse.mybir as mybir
import concourse.tile as tile
from concourse import bacc
from concourse.bass import ds
from concourse.masks import make_identity

dt = mybir.dt
F32 = dt.float32
BF16 = dt.bfloat16
FP8 = dt.float8e4
I8 = dt.int8
AF = mybir.ActivationFunctionType
ALU = mybir.AluOpType
AX = mybir.AxisListType
DR = mybir.MatmulPerfMode.DoubleRow

D = 256      # model dim
DH = 64      # head dim
HPC = 2      # heads per core
KD = D // 128
LN_EPS = 1e-5
SM_SCALE = 0.125  # 1/sqrt(DH)
T_SCALE = 16.0

# Schraudolph-style exp constants (fp8e4m3 bit domain, rint semantics):
#   i8 = rint(A8C * s + B8C);  bitcast_e4m3(i8) ~= exp(s * SM_SCALE)
A8C = 8.0 / np.log(2.0) * SM_SCALE
B8C = 55.64

STAG = 8          # head-1 chunk-slot stagger (spare P slots)
DVE_FRACA = 0.33  # DVE share of phase-A exp slots (overlaps setup)
DVE_FRAC0 = 0.46  # DVE share of remaining head-0 slots
DVE_FRAC1 = 0.42  # DVE share of head-1 slots


def _slot_plan(segments):
    """Per-slot engine assignment from (count, dve_frac, warm) segments;
    True = DVE."""
    plan = []
    for n, frac, warm in segments:
        for i in range(n):
            plan.append(i >= warm and
                        int((i + 1) * frac) > int(i * frac))
    return plan


def build_kernel(nc, N=4096, replica_groups=((0, 1, 2, 3), (4, 5, 6, 7)),
                 p_dtype=FP8, collective=True):
    NC = N // 128
    replica_groups = [list(g) for g in replica_groups]
    G = len(replica_groups[0]) if collective else 4

    x = nc.dram_tensor("x", [N, D], F32, kind="ExternalInput")
    wq_d = nc.dram_tensor("wq", [D, HPC * DH], F32, kind="ExternalInput")
    wk_d = nc.dram_tensor("wk", [D, HPC * DH], F32, kind="ExternalInput")
    wv_d = nc.dram_tensor("wv", [D, HPC * DH], F32, kind="ExternalInput")
    wo_d = nc.dram_tensor("wo", [HPC * DH, D], F32, kind="ExternalInput")
    gamma_d = nc.dram_tensor("gamma2", [1, HPC * DH], F32, kind="ExternalInput")
    beta_d = nc.dram_tensor("beta2", [1, HPC * DH], F32, kind="ExternalInput")
    cf_d = nc.dram_tensor("cf", [1, HPC * 4], F32, kind="ExternalInput")
    out_d = nc.dram_tensor("out", [N // G, D], BF16, kind="ExternalOutput")

    with tile.TileContext(nc) as tc:
        _body(nc, tc, N, NC, replica_groups,
              x, wq_d, wk_d, wv_d, wo_d, gamma_d, beta_d, cf_d, out_d,
              collective)
    return nc


def _body(nc, tc, N, NC, replica_groups,
          x, wq_d, wk_d, wv_d, wo_d, gamma_d, beta_d, cf_d, out_d,
          collective=True):
    HN = NC * N            # flat columns per head (chunk-major)

    with (
        tc.tile_pool(name="persist", bufs=1) as pp,
        tc.tile_pool(name="dram", bufs=1, space="DRAM") as dram,
    ):
        # ---------- persistent tiles ----------
        ident_b = pp.tile([128, 128], BF16)

        cbc = pp.tile([128, 8], F32)       # coeffs broadcast [h*4 + k]
        cbc16 = pp.tile([128, 8], F32)     # coeffs / T_SCALE

        P = pp.tile([128, NC, N], FP8)     # current head's P^T (chunk-major)
        QT8 = pp.tile([64, 2, N], FP8)     # zero-padded pair layout
        KT8 = pp.tile([64, 2, N], FP8)
        vln = pp.tile([128, NC, HPC, 64], BF16)
        vln8 = pp.tile([128, NC, HPC, 80], FP8)   # [v_ln | 1 | 0-pad]
        t8a = pp.tile([128, NC, 64], FP8)
        t8b = pp.tile([128, NC, 64], FP8)
        t8s = (t8a, t8b)
        yt = pp.tile([128, NC, HPC, 64], BF16)
        rinv = pp.tile([128, NC, 1], F32)
        wo_s = pp.tile([64, HPC, 256], BF16)
        gbc = pp.tile([128, 1, HPC, 64], F32)
        bbc = pp.tile([128, 1, HPC, 64], F32)

        qk8_dr = dram.tile([2, 64, N], FP8)       # head-1 Q^T/K^T (real half)
        xb_dr = dram.tile([N, D], BF16)
        bounce_in = dram.tile([N, D], BF16)

        nc.vector.memset(QT8[:, 1, :], 0.0)
        nc.vector.memset(KT8[:, 1, :], 0.0)
        nc.vector.memset(vln8[:, :, :, 64:65], 1.0)
        nc.vector.memset(vln8[:, :, :, 65:80], 0.0)
        make_identity(nc, ident_b)

        # ================= main: exp slots + applications =================
        SW = 1024
        nA = (N // 512) * NC          # phase-A: q-strip 0 in 512-wide slots
        nB = 3 * NC                   # phase-B: q-strips 1-3, 1024-wide
        n_h1 = HN // SW
        total_slots = nA + nB + n_h1
        Pflat = P[:].rearrange("p c n -> p (c n)")

        slot_plan = _slot_plan([(nA, DVE_FRACA, 8),
                                (nB, DVE_FRAC0, 0),
                                (n_h1, DVE_FRAC1, 0)])

        def consume(slot_ap, pieces, gslot):
            off = 0
            for dst_ap, w in pieces:
                src = slot_ap[:, ds(off, w)]
                if slot_plan[gslot]:
                    nc.vector.tensor_scalar(
                        dst_ap.bitcast(I8), src, A8C, B8C, ALU.mult, ALU.add)
                else:
                    nc.scalar.activation(dst_ap, src, AF.Exp, scale=SM_SCALE)
                off += w

        def prod_gen(ring, slot_w, slots, gslot0, pw):
            """Produce + exp P slots. `slots` is a list of head-flat start
            columns (width slot_w each); pw(flat_col, width) -> [(AP, w)]."""
            for s, col in enumerate(slots):
                w = slot_w
                sl = ring.tile([128, slot_w], F32, tag="sl")
                for q in range(w // 512):
                    fc = col + q * 512
                    mc, qc = fc // N, fc % N
                    nc.tensor.matmul(
                        sl[:, ds(q * 512, 512)],
                        KT8[:, :, ds(mc * 128, 128)],
                        QT8[:, :, ds(qc, 512)],
                        start=True, stop=True, perf_mode=DR)
                consume(sl, pw(col, w), gslot0 + s)
                yield

        # ================= setup =================
        with (
            tc.tile_pool(name="setup", bufs=1) as sp,
            tc.tile_pool(name="xio", bufs=2) as xp,
            tc.tile_pool(name="qk_psum", bufs=2, space="PSUM") as qpp,
            tc.tile_pool(name="v_psum", bufs=1, space="PSUM") as vpp,
            tc.tile_pool(name="xt_psum", bufs=1, space="PSUM") as xtp,
            tc.tile_pool(name="qk_st", bufs=2) as qst,
            tc.tile_pool(name="vln_st", bufs=1) as vst,
        ):
            # constants
            c1 = sp.tile([1, 8], F32)
            nc.sync.dma_start(c1[:], cf_d.ap())
            nc.gpsimd.partition_broadcast(cbc[:], c1[:])
            nc.vector.tensor_scalar_mul(cbc16[:], cbc[:], 1.0 / T_SCALE)
            g1 = sp.tile([1, 128], F32)
            nc.sync.dma_start(g1[:], gamma_d.ap())
            nc.gpsimd.partition_broadcast(
                gbc[:].rearrange("p o a b -> p (o a b)"), g1[:])
            b1 = sp.tile([1, 128], F32)
            nc.sync.dma_start(b1[:], beta_d.ap())
            nc.gpsimd.partition_broadcast(
                bbc[:].rearrange("p o a b -> p (o a b)"), b1[:])

            # weights -> SBUF bf16 (staged through one f32 tile)
            wsb = sp.tile([128, 3, KD, 128], BF16)
            wstg = sp.tile([128, KD, 128], F32)
            for wi, w_d in enumerate((wq_d, wk_d, wv_d)):
                nc.sync.dma_start(
                    wstg[:], w_d.ap().rearrange("(o p) m -> p o m", p=128))
                nc.vector.tensor_copy(wsb[:, wi], wstg[:])
            wo_f = sp.tile([64, HPC, 256], F32)
            nc.scalar.dma_start(
                wo_f[:], wo_d.ap().rearrange("(h d) m -> d h m", h=HPC))
            nc.vector.tensor_copy(wo_s[:], wo_f[:])

            # ---- per-quarter (1024 rows): x -> x^T -> Q/K proj -> V/LN ----
            QW = 1024
            x_r = x.ap().rearrange("(o p) d -> p o d", p=128)

            vsb = sp.tile([128, NC, HPC, 64], BF16)
            mu = sp.tile([128, NC, HPC, 1], F32)
            var = sp.tile([128, NC, HPC, 1], F32)
            rstd = sp.tile([128, NC, HPC, 1], F32)
            s1 = sp.tile([128, 4, HPC], F32)
            s2 = sp.tile([128, 4, HPC], F32)

            def emit_quarter(q):
                # x rows -> x^T quarter (double-buffered)
                xT = xp.tile([128, KD, QW], BF16, tag="xT")
                for g in range(4):
                    xf = xp.tile([128, 2, D], F32, tag="xf")
                    nc.sync.dma_start(xf[:], x_r[:, ds(q * 8 + g * 2, 2), :])
                    xb = xp.tile([128, 2, D], BF16, tag="xb")
                    nc.gpsimd.tensor_copy(xb[:], xf[:])
                    tps = xtp.tile([128, 2, KD, 128], BF16, tag="tps")
                    for j in range(2):
                        for kd in range(KD):
                            nc.tensor.transpose(
                                tps[:, j, kd, :],
                                xb[:, j, ds(kd * 128, 128)], ident_b[:])
                    nc.scalar.activation(
                        xT[:, :, ds(g * 256, 256)],
                        tps[:].rearrange("p j k c -> p k j c"), AF.Copy)
                # Q/K projections, head 0 -> SBUF, head 1 -> DRAM
                for h in range(2):
                    for qi in range(2):
                        for nb in range(2):
                            gb = q * 2 + nb
                            ps = qpp.tile([64, 512], F32, tag="psq")
                            for kd in range(KD):
                                nc.tensor.matmul(
                                    ps[:],
                                    wsb[:, qi, kd, ds(h * 64, 64)],
                                    xT[:, kd, ds(nb * 512, 512)],
                                    start=(kd == 0), stop=(kd == KD - 1))
                            if h == 0:
                                dst = QT8 if qi == 0 else KT8
                                nc.scalar.activation(
                                    dst[:, 0, ds(gb * 512, 512)], ps[:],
                                    AF.Copy)
                            else:
                                stg = qst.tile([64, 512], FP8, tag="stg")
                                nc.scalar.activation(stg[:], ps[:], AF.Copy)
                                nc.sync.dma_start(
                                    qk8_dr[qi, :, ds(gb * 512, 512)], stg[:])
                # V projection + LN stats (two 4-chunk groups per quarter)
                for lg in range(2):
                    cg = q * 2 + lg
                    vps = vpp.tile([128, 4, 128], F32, tag="vps")
                    for j in range(4):
                        for kd in range(KD):
                            nc.tensor.matmul(
                                vps[:, j, :],
                                xT[:, kd, ds((lg * 4 + j) * 128, 128)],
                                wsb[:, 2, kd, :],
                                start=(kd == 0), stop=(kd == KD - 1))
                    vsl = vsb[:, ds(cg * 4, 4), :, :]
                    nc.vector.tensor_copy(
                        vsl, vps[:].rearrange("p j (h d) -> p j h d", h=HPC))
                    nc.vector.tensor_reduce(
                        s1[:].rearrange("p a b -> p (a b)"), vsl, axis=AX.X,
                        op=ALU.add)
                    sqs = vst.tile([128, 4, HPC, 64], BF16, tag="sqs")
                    nc.gpsimd.tensor_tensor(sqs[:], vsl, vsl, ALU.mult)
                    nc.vector.tensor_reduce(
                        s2[:].rearrange("p a b -> p (a b)"), sqs[:], axis=AX.X,
                        op=ALU.add)
                    muf = mu[:, ds(cg * 4, 4)].rearrange(
                        "p a b c -> p (a b c)")
                    varf = var[:, ds(cg * 4, 4)].rearrange(
                        "p a b c -> p (a b c)")
                    s1f = s1[:].rearrange("p a b -> p (a b)")
                    s2f = s2[:].rearrange("p a b -> p (a b)")
                    nc.vector.tensor_scalar_mul(muf, s1f, 1.0 / 64.0)
                    nc.vector.scalar_tensor_tensor(
                        varf, muf, -1.0, muf, ALU.mult, ALU.mult)   # -mu^2
                    nc.vector.scalar_tensor_tensor(
                        varf, s2f, 1.0 / 64.0, varf, ALU.mult, ALU.add)
                    # chain part 1 (rstd-independent): va = (v - mu) * gamma
                    sh = [128, 4, HPC, 64]
                    va = vsb[:, ds(cg * 4, 4), :, :]
                    nc.gpsimd.tensor_tensor(
                        va, va, mu[:, ds(cg * 4, 4)].broadcast_to(sh),
                        ALU.subtract)
                    nc.gpsimd.tensor_tensor(
                        va, va, gbc[:].broadcast_to(sh), ALU.mult)

            def vln_chain_units():
                # rstd = exp(-0.5 * ln(var + eps)), one Ln + one Exp total
                varf = var[:].rearrange("p a b c -> p (a b c)")
                nc.vector.tensor_scalar_add(varf, varf, LN_EPS)
                nc.scalar.activation(varf, varf, AF.Ln)
                nc.scalar.activation(
                    rstd[:].rearrange("p a b c -> p (a b c)"), varf,
                    AF.Exp, scale=-0.5)
                yield
                # part 2: vln = va * rstd + beta, mostly on idle Pool
                for q in range(NC // 4):
                    sh = [128, 4, HPC, 64]
                    va = vsb[:, ds(q * 4, 4), :, :]
                    vsl = vln[:, ds(q * 4, 4), :, :]
                    eng = nc.gpsimd if q % 4 != 3 else nc.vector
                    eng.tensor_tensor(
                        vsl, va, rstd[:, ds(q * 4, 4)].broadcast_to(sh),
                        ALU.mult)
                    eng.tensor_tensor(
                        vsl, vsl, bbc[:].broadcast_to(sh), ALU.add)
                    eng.tensor_copy(vln8[:, ds(q * 4, 4), :, 0:64], vsl)
                    yield
                    # y init for this group: y_h = c0_h * vln_h
                    for h in range(HPC):
                        nc.vector.tensor_scalar_mul(
                            yt[:, ds(q * 4, 4), h, :],
                            vln[:, ds(q * 4, 4), h, :], cbc[:, ds(h * 4, 1)])
                    yield

            emit_quarter(0)
            # phase-A: q-strip 0 exp slots overlap the rest of setup
            slotsA = [mc * N + hh * 512 for mc in range(NC)
                      for hh in range(2)]
            with tc.tile_pool(name="ring0a", bufs=4, space="PSUM") as ring0a:
                gA = prod_gen(ring0a, 512, slotsA, 0,
                              lambda c, w: [(Pflat[:, ds(c, w)], w)])
                done_a = False
                for q in (1, 2, 3):
                    emit_quarter(q)
                    for _ in range(14):
                        if next(gA, "end") == "end":
                            done_a = True
                            break
                gC = vln_chain_units()
                done_c = False
                while not (done_a and done_c):
                    if not done_a:
                        for _ in range(2):
                            if next(gA, "end") == "end":
                                done_a = True
                                break
                    if not done_c:
                        done_c = next(gC, "end") == "end" 


        # head-1 chunk c -> spare slot c (c < STAG) else P slot c - STAG
        def app_lhsT(h, kp, j, P_spare):
            """Stationary P pair (chunks 2kp, 2kp+1) for q-window j."""
            if h == 0:
                return P[:, ds(2 * kp, 2), ds(j * 128, 128)]
            c = 2 * kp
            if c + 2 <= STAG:
                return P_spare[:, ds(c, 2), ds(j * 128, 128)]
            return P[:, ds(c - STAG, 2), ds(j * 128, 128)]

        def h1_pw(P_spare):
            Psf = P_spare[:].rearrange("p c n -> p (c n)")
            sb = STAG * N

            def pw(col, w):
                # [col, col+w) relative to head start; spare covers first sb
                if col + w <= sb:
                    return [(Psf[:, ds(col, w)], w)]
                if col >= sb:
                    return [(Pflat[:, ds(col - sb, w)], w)]
                w0 = sb - col
                return [(Psf[:, ds(col, w0)], w0),
                        (Pflat[:, ds(0, w - w0)], w - w0)]
            return pw

        def apps_gen(h, accp, P_spare, apps=(1, 2, 3), gsz=8):
            """Graph-filter applications for head h (natural-layout form).
            Yields ("grp", app, last_chunk) after each group epilogue."""
            acc_w = 512
            for app in apps:
                if app == 1:
                    groups = [list(range(g * 7, min(NC, g * 7 + 7)))
                              for g in range(5)]
                    W = 65
                else:
                    groups = [list(range(g * gsz, g * gsz + gsz))
                              for g in range(NC // gsz)]
                    W = 64
                for cl in groups:
                    L = len(cl)
                    t8r, t8w = t8s[app % 2], t8s[(app + 1) % 2]
                    acc = accp.tile([128, acc_w], F32, tag="acc")
                    for ji, j in enumerate(cl):
                        for kp in range(NC // 2):
                            rhs = (vln8[:, ds(2 * kp, 2), h, 0:W] if app == 1
                                   else t8r[:, ds(2 * kp, 2), :])
                            nc.tensor.matmul(
                                acc[:, ds(ji * W, W)],
                                app_lhsT(h, kp, j, P_spare), rhs,
                                start=(kp == 0), stop=(kp == NC // 2 - 1),
                                perf_mode=DR)
                        yield
                    accg = acc[:, 0:L * W].rearrange("p (a b) -> p a b", b=W)
                    c0 = cl[0]
                    if app == 1:
                        nc.vector.reciprocal(
                            rinv[:, ds(c0, L), :], accg[:, :, 64:65])
                    # t8 = (u * scale) * (1/r)
                    nc.vector.scalar_tensor_tensor(
                        t8w[:, ds(c0, L), :], accg[:, :, 0:64],
                        T_SCALE if app == 1 else 1.0,
                        rinv[:, ds(c0, L), :].broadcast_to([128, L, 64]),
                        ALU.mult, ALU.mult)
                    if h == 1 and app == 3:
                        # per-group y so outproj can chase app3
                        nc.vector.scalar_tensor_tensor(
                            yt[:, ds(c0, L), h, :], t8w[:, ds(c0, L), :],
                            cbc16[:, ds(h * 4 + app, 1)],
                            yt[:, ds(c0, L), h, :], ALU.mult, ALU.add)
                    yield ("grp", app, cl[-1])
                if not (h == 1 and app == 3):
                    # y += (c_app / T_SCALE) * t8, batched off the group chain
                    nc.vector.scalar_tensor_tensor(
                        yt[:, :, h, :], t8s[(app + 1) % 2][:],
                        cbc16[:, ds(h * 4 + app, 1)],
                        yt[:, :, h, :], ALU.mult, ALU.add)
                yield ("app", app, NC - 1)

        def make_outproj(opp, ost, obt):
            """Output projection in 2-chunk units, stage-skewed emission so
            PE never head-of-line blocks on the yst copy."""
            state = {"ty": 0, "tail": 0, "yst_tiles": {}}

            def emit_ty(u):
                c0 = u * 2
                ty = opp.tile([64, 2, HPC, 128], BF16, tag="ty")
                for j in range(2):
                    for h in range(HPC):
                        nc.tensor.transpose(
                            ty[:, j, h, :], yt[:, c0 + j, h, :], ident_b[:])
                yst = ost.tile([64, 2, HPC, 128], BF16, tag="yst")
                nc.scalar.activation(yst[:], ty[:], AF.Copy)
                state["yst_tiles"][u] = yst

            def emit_tail(u):
                c0 = u * 2
                yst = state["yst_tiles"].pop(u)
                po = opp.tile([128, 2, 256], F32, tag="po")
                for j in range(2):
                    for h in range(HPC):
                        nc.tensor.matmul(
                            po[:, j, :], yst[:, j, h, :],
                            wo_s[:, h, :],
                            start=(h == 0), stop=(h == HPC - 1))
                ob = obt.tile([128, 2, 256], BF16, tag="ob")
                nc.vector.tensor_copy(ob[:], po[:])
                nc.sync.dma_start(
                    bounce_in[:].rearrange("(c p) d -> p c d", p=128)[
                        :, ds(c0, 2), :],
                    ob[:])

            def pump(ready_units):
                while state["ty"] < ready_units:
                    emit_ty(state["ty"])
                    state["ty"] += 1
                    while state["tail"] < state["ty"] - 1:
                        emit_tail(state["tail"])
                        state["tail"] += 1

            def flush():
                pump(NC // 2)
                while state["tail"] < NC // 2:
                    emit_tail(state["tail"])
                    state["tail"] += 1

            return pump, flush, state

        with tc.tile_pool(name="spare", bufs=1) as spare_pool:
            P_spare = spare_pool.tile([128, STAG, N], FP8)

            # ---- head 0 phase-B slots: q-strips 1-3, strip-major ----
            # QT8 strip reloads interleave (strip q's Q cols are only read
            # by strip q's own matmuls); KT8 reloads after its last reader.
            nc.sync.dma_start(QT8[:, 0, 0:SW], qk8_dr[0][:, 0:SW])
            slotsB = [mc * N + q * SW for q in (1, 2, 3) for mc in range(NC)]
            with tc.tile_pool(name="ring0", bufs=4, space="PSUM") as ring0:
                g0 = prod_gen(ring0, SW, slotsB, nA,
                              lambda c, w: [(Pflat[:, ds(c, w)], w)])
                for i, _ in enumerate(g0):
                    if i % NC == NC - 1:
                        q = 1 + i // NC
                        nc.sync.dma_start(QT8[:, 0, ds(q * SW, SW)],
                                          qk8_dr[0][:, ds(q * SW, SW)])
            nc.sync.dma_start(KT8[:, 0, :], qk8_dr[1])

            # ---- head 1 slots (1536-wide ring) + head-0 apps interleaved ----
            with (
                tc.tile_pool(name="ring1", bufs=3, space="PSUM") as ring1,
                tc.tile_pool(name="acc0", bufs=2, space="PSUM") as acc0,
            ):
                slots1 = [i * SW for i in range(n_h1)]
                g1 = prod_gen(ring1, SW, slots1, nA + nB, h1_pw(P_spare))
                ga = apps_gen(0, acc0, P_spare)
                done_p = done_a = False
                while not (done_p and done_a):
                    if not done_p:
                        done_p = next(g1, "end") == "end"
                    if not done_a:
                        for _ in range(8):
                            if next(ga, "end") == "end":
                                done_a = True
                                break

            # ---- tail: head-1 apps + output projection (chasing app3) ----
            with (
                tc.tile_pool(name="acc1", bufs=4, space="PSUM") as acc1,
                tc.tile_pool(name="op_psum", bufs=2, space="PSUM") as opp,
                tc.tile_pool(name="o_st", bufs=2) as ost,
                tc.tile_pool(name="ob_st", bufs=2) as obt,
            ):
                for _ in apps_gen(1, acc1, P_spare, apps=(1, 2)):
                    pass
                pump, flush, op_state = make_outproj(opp, ost, obt)
                out_r = out_d.ap().rearrange("(c p) d -> p c d", p=128)
                bnc_r = bounce_in[:].rearrange("(c p) d -> p c d", p=128)
                out_done = [0]

                def chase_out(units):
                    # out rows 0:1024 = chunks 0:8 = outproj units 0:4
                    if collective:
                        return
                    lim = min(units, 4)
                    while out_done[0] < lim:
                        u = out_done[0]
                        nc.sync.dma_start(out_r[:, ds(u * 2, 2), :],
                                          bnc_r[:, ds(u * 2, 2), :])
                        out_done[0] += 1

                for u in apps_gen(1, acc1, P_spare, apps=(3,), gsz=4):
                    if isinstance(u, tuple) and u[0] == "grp":
                        pump((u[2] + 1) // 2)
                        chase_out(op_state["tail"])
                flush()
                chase_out(4)

        # ---------- ReduceScatter (bf16) over the batch group + output ------
        if not collective:
            # output rows were already DMA'd piecewise by chase_out
            return
        G = len(replica_groups[0])
        bounce_out = dram.tile([N // G, D], BF16)
        nc.gpsimd.collective_compute(
            "ReduceScatter", ALU.add, replica_groups=replica_groups,
            ins=[bounce_in.opt()], outs=[bounce_out.opt()])
        nc.sync.dma_start(out_d.ap(), bounce_out[:])


# ----------------------------------------------------------------------------
# host-side entry point
# ----------------------------------------------------------------------------

_CACHED = {}


def _get_compiled(N=4096, n_cores=8, p_dtype=FP8):
    key = (N, n_cores, p_dtype)
    if key not in _CACHED:
        groups = [list(range(g * 4, g * 4 + 4)) for g in range(2)] \
            if n_cores == 8 else [list(range(n_cores))]
        nc = bacc.Bacc("TRN2", target_bir_lowering=False, debug=False,
                       num_devices=n_cores)
        build_kernel(nc, N=N, replica_groups=groups, p_dtype=p_dtype)
        nc.compile()
        _CACHED[key] = nc
    return _CACHED[key]


def make_in_maps(x, Wq, Wk, Wv, Wo, gamma, beta, coeffs, n_cores=8):
    """Shard full inputs into per-core input maps (batch + head-pair)."""
    x = np.asarray(x, np.float32)
    Wq = np.asarray(Wq, np.float32)
    Wk = np.asarray(Wk, np.float32)
    Wv = np.asarray(Wv, np.float32)
    Wo = np.asarray(Wo, np.float32)
    gamma = np.asarray(gamma, np.float32)
    beta = np.asarray(beta, np.float32)
    coeffs = np.asarray(coeffs, np.float32)
    g2 = np.concatenate([gamma, gamma]).reshape(1, 128).copy()
    b2 = np.concatenate([beta, beta]).reshape(1, 128).copy()
    in_maps = []
    for core in range(n_cores):
        b = core // 4 if n_cores == 8 else 0
        hp = core % 4 if n_cores == 8 else core
        cols = slice(hp * 128, (hp + 1) * 128)
        in_maps.append({
            "x": np.ascontiguousarray(x[b]),
            "wq": np.ascontiguousarray(Wq[:, cols]),
            "wk": np.ascontiguousarray(Wk[:, cols]),
            "wv": np.ascontiguousarray(Wv[:, cols]),
            "wo": np.ascontiguousarray(Wo[cols, :]),
            "gamma2": g2,
            "beta2": b2,
            "cf": np.ascontiguousarray(coeffs[2 * hp: 2 * hp + 2].reshape(1, 8)),
        })
    return in_maps


def kernel(x, Wq, Wk, Wv, Wo, gamma, beta, coeffs, trace=False):
    from concourse.bass_utils import run_bass_kernel_spmd

    n_cores = 8
    nc = _get_compiled(4096, n_cores)
    in_maps = make_in_maps(x, Wq, Wk, Wv, Wo, gamma, beta, coeffs, n_cores)
    res = run_bass_kernel_spmd(nc, in_maps, core_ids=list(range(n_cores)),
                               trace=trace)
    # each core returns its ReduceScatter shard: rank k of a batch group
    # holds rows [k*N/4, (k+1)*N/4) of that batch's output
    N = 4096
    out = np.empty((2, N, 256), np.float32)
    for b, cores in enumerate([[0, 1, 2, 3], [4, 5, 6, 7]]):
        for k, c in enumerate(cores):
            shard = np.asarray(res.results[c]["out"]).astype(np.float32)
            out[b, k * (N // 4):(k + 1) * (N // 4)] = shard
    if trace:
        kernel.last_result = res
    return out


# revision 61
# speedup vs baseline: 1.6504x; 1.0015x over previous
"""AGF attention (graph-filter attention) distributed Bass kernel for 8 TRN2
cores.

Sharding: batch x head-pair (data + head parallel). Core i handles batch
b = i//4 and heads {2*(i%4), 2*(i%4)+1}. Each core computes its partial
output projection (summed over its 2 heads); a bf16 ReduceScatter over the
4 cores of each batch produces row shards of that batch's [N, D] output,
which the host concatenates.

v3 design (per core), engine-balanced around the exp() bottleneck:
  - S^T is produced by fp8 DoubleRow matmuls (Q^T/K^T quantized to fp8 in a
    zero-padded [64, 2, N] pair layout, so the PSUM->SBUF projection copies
    double as the quantization) into a small PSUM ring.
  - P = exp(S^T/8) is split between ACT (native Exp -> fp8) and DVE (a
    Schraudolph-style exp: i8 = rint(log2e/8-scaled logit + bias) written
    through an int8 bitcast of the fp8 P tile; the int8 IS the fp8 bit
    pattern). This roughly doubles exp throughput, the kernel bottleneck.
  - P stays fully SBUF-resident per head ([128, NC, N] fp8); head 1 reuses
    head 0's chunk slots (stagger via a spare-slot tile) with no DRAM spill.
  - Graph-filter applications keep P as the *stationary* operand:
    u[q, d] = sum_k P^T[k, q] t[k, d] comes out in natural layout (no
    transpose-back), out-free = 64/65 cols, fp8 DR. app1 carries a ones
    column to produce the softmax row-sums.
  - Epilogues (t requant + y accumulate) are batched DVE ops; LayerNorm
    elementwise chain and misc copies run on the otherwise-idle GPSIMD.
  - x^T is produced by xbar DMA transposes (DRAM bf16 bounce), not PE.
"""

import numpy as np

import concourse.mybir as mybir
import concourse.tile as tile
from concourse import bacc
from concourse.bass import ds
from concourse.masks import make_identity

dt = mybir.dt
F32 = dt.float32
BF16 = dt.bfloat16
FP8 = dt.float8e4
I8 = dt.int8
AF = mybir.ActivationFunctionType
ALU = mybir.AluOpType
AX = mybir.AxisListType
DR = mybir.MatmulPerfMode.DoubleRow

D = 256      # model dim
DH = 64      # head dim
HPC = 2      # heads per core
KD = D // 128
LN_EPS = 1e-5
SM_SCALE = 0.125  # 1/sqrt(DH)
T_SCALE = 16.0

# Schraudolph-style exp constants (fp8e4m3 bit domain, rint semantics):
#   i8 = rint(A8C * s + B8C);  bitcast_e4m3(i8) ~= exp(s * SM_SCALE)
A8C = 8.0 / np.log(2.0) * SM_SCALE
B8C = 55.64

STAG = 8          # head-1 chunk-slot stagger (spare P slots)
DVE_FRACA = 0.33  # DVE share of phase-A exp slots (overlaps setup)
DVE_FRAC0 = 0.46  # DVE share of remaining head-0 slots
DVE_FRAC1 = 0.42  # DVE share of head-1 slots


def _slot_plan(segments):
    """Per-slot engine assignment from (count, dve_frac, warm) segments;
    True = DVE."""
    plan = []
    for n, frac, warm in segments:
        for i in range(n):
            plan.append(i >= warm and
                        int((i + 1) * frac) > int(i * frac))
    return plan


def build_kernel(nc, N=4096, replica_groups=((0, 1, 2, 3), (4, 5, 6, 7)),
                 p_dtype=FP8, collective=True):
    NC = N // 128
    replica_groups = [list(g) for g in replica_groups]
    G = len(replica_groups[0]) if collective else 4

    x = nc.dram_tensor("x", [N, D], F32, kind="ExternalInput")
    wq_d = nc.dram_tensor("wq", [D, HPC * DH], F32, kind="ExternalInput")
    wk_d = nc.dram_tensor("wk", [D, HPC * DH], F32, kind="ExternalInput")
    wv_d = nc.dram_tensor("wv", [D, HPC * DH], F32, kind="ExternalInput")
    wo_d = nc.dram_tensor("wo", [HPC * DH, D], F32, kind="ExternalInput")
    gamma_d = nc.dram_tensor("gamma2", [1, HPC * DH], F32, kind="ExternalInput")
    beta_d = nc.dram_tensor("beta2", [1, HPC * DH], F32, kind="ExternalInput")
    cf_d = nc.dram_tensor("cf", [1, HPC * 4], F32, kind="ExternalInput")
    out_d = nc.dram_tensor("out", [N // G, D], BF16, kind="ExternalOutput")

    with tile.TileContext(nc) as tc:
        _body(nc, tc, N, NC, replica_groups,
              x, wq_d, wk_d, wv_d, wo_d, gamma_d, beta_d, cf_d, out_d,
              collective)
    return nc


def _body(nc, tc, N, NC, replica_groups,
          x, wq_d, wk_d, wv_d, wo_d, gamma_d, beta_d, cf_d, out_d,
          collective=True):
    HN = NC * N            # flat columns per head (chunk-major)

    with (
        tc.tile_pool(name="persist", bufs=1) as pp,
        tc.tile_pool(name="dram", bufs=1, space="DRAM") as dram,
    ):
        # ---------- persistent tiles ----------
        ident_b = pp.tile([128, 128], BF16)

        cbc = pp.tile([128, 8], F32)       # coeffs broadcast [h*4 + k]
        cbc16 = pp.tile([128, 8], F32)     # coeffs / T_SCALE

        P = pp.tile([128, NC, N], FP8)     # current head's P^T (chunk-major)
        QT8 = pp.tile([64, 2, N], FP8)     # zero-padded pair layout
        KT8 = pp.tile([64, 2, N], FP8)
        vln = pp.tile([128, NC, HPC, 64], BF16)
        vln8 = pp.tile([128, NC, HPC, 80], FP8)   # [v_ln | 1 | 0-pad]
        t8a = pp.tile([128, NC, 64], FP8)
        t8b = pp.tile([128, NC, 64], FP8)
        t8s = (t8a, t8b)
        yt = pp.tile([128, NC, HPC, 64], BF16)
        rinv = pp.tile([128, NC, 1], F32)
        wo_s = pp.tile([64, HPC, 256], BF16)
        gbc = pp.tile([128, 1, HPC, 64], F32)
        bbc = pp.tile([128, 1, HPC, 64], F32)

        qk8_dr = dram.tile([2, 64, N], FP8)       # head-1 Q^T/K^T (real half)
        xb_dr = dram.tile([N, D], BF16)
        bounce_in = dram.tile([N, D], BF16)

        nc.vector.memset(QT8[:, 1, :], 0.0)
        nc.vector.memset(KT8[:, 1, :], 0.0)
        nc.vector.memset(vln8[:, :, :, 64:65], 1.0)
        nc.vector.memset(vln8[:, :, :, 65:80], 0.0)
        make_identity(nc, ident_b)

        # ================= main: exp slots + applications =================
        SW = 1024
        nA = (N // 512) * NC          # phase-A: q-strip 0 in 512-wide slots
        nB = 3 * NC                   # phase-B: q-strips 1-3, 1024-wide
        n_h1 = HN // SW
        total_slots = nA + nB + n_h1
        Pflat = P[:].rearrange("p c n -> p (c n)")

        slot_plan = _slot_plan([(nA, DVE_FRACA, 4),
                                (nB, DVE_FRAC0, 0),
                                (n_h1, DVE_FRAC1, 0)])

        def consume(slot_ap, pieces, gslot):
            off = 0
            for dst_ap, w in pieces:
                src = slot_ap[:, ds(off, w)]
                if slot_plan[gslot]:
                    nc.vector.tensor_scalar(
                        dst_ap.bitcast(I8), src, A8C, B8C, ALU.mult, ALU.add)
                else:
                    nc.scalar.activation(dst_ap, src, AF.Exp, scale=SM_SCALE)
                off += w

        def prod_gen(ring, slot_w, slots, gslot0, pw):
            """Produce + exp P slots. `slots` is a list of head-flat start
            columns (width slot_w each); pw(flat_col, width) -> [(AP, w)]."""
            for s, col in enumerate(slots):
                w = slot_w
                sl = ring.tile([128, slot_w], F32, tag="sl")
                for q in range(w // 512):
                    fc = col + q * 512
                    mc, qc = fc // N, fc % N
                    nc.tensor.matmul(
                        sl[:, ds(q * 512, 512)],
                        KT8[:, :, ds(mc * 128, 128)],
                        QT8[:, :, ds(qc, 512)],
                        start=True, stop=True, perf_mode=DR)
                consume(sl, pw(col, w), gslot0 + s)
                yield

        # ================= setup =================
        with (
            tc.tile_pool(name="setup", bufs=1) as sp,
            tc.tile_pool(name="xio", bufs=2) as xp,
            tc.tile_pool(name="qk_psum", bufs=2, space="PSUM") as qpp,
            tc.tile_pool(name="v_psum", bufs=1, space="PSUM") as vpp,
            tc.tile_pool(name="xt_psum", bufs=1, space="PSUM") as xtp,
            tc.tile_pool(name="qk_st", bufs=2) as qst,
            tc.tile_pool(name="vln_st", bufs=1) as vst,
        ):
            # constants
            c1 = sp.tile([1, 8], F32)
            nc.sync.dma_start(c1[:], cf_d.ap())
            nc.gpsimd.partition_broadcast(cbc[:], c1[:])
            nc.vector.tensor_scalar_mul(cbc16[:], cbc[:], 1.0 / T_SCALE)
            g1 = sp.tile([1, 128], F32)
            nc.sync.dma_start(g1[:], gamma_d.ap())
            nc.gpsimd.partition_broadcast(
                gbc[:].rearrange("p o a b -> p (o a b)"), g1[:])
            b1 = sp.tile([1, 128], F32)
            nc.sync.dma_start(b1[:], beta_d.ap())
            nc.gpsimd.partition_broadcast(
                bbc[:].rearrange("p o a b -> p (o a b)"), b1[:])

            # weights -> SBUF bf16 (staged through one f32 tile)
            wsb = sp.tile([128, 3, KD, 128], BF16)
            wstg = sp.tile([128, KD, 128], F32)
            for wi, w_d in enumerate((wq_d, wk_d, wv_d)):
                nc.sync.dma_start(
                    wstg[:], w_d.ap().rearrange("(o p) m -> p o m", p=128))
                nc.vector.tensor_copy(wsb[:, wi], wstg[:])
            wo_f = sp.tile([64, HPC, 256], F32)
            nc.scalar.dma_start(
                wo_f[:], wo_d.ap().rearrange("(h d) m -> d h m", h=HPC))
            nc.vector.tensor_copy(wo_s[:], wo_f[:])

            # ---- per-quarter (1024 rows): x -> x^T -> Q/K proj -> V/LN ----
            QW = 1024
            x_r = x.ap().rearrange("(o p) d -> p o d", p=128)

            vsb = sp.tile([128, NC, HPC, 64], BF16)
            mu = sp.tile([128, NC, HPC, 1], F32)
            var = sp.tile([128, NC, HPC, 1], F32)
            rstd = sp.tile([128, NC, HPC, 1], F32)
            s1 = sp.tile([128, 4, HPC], F32)
            s2 = sp.tile([128, 4, HPC], F32)

            def emit_quarter(q):
                # x rows -> x^T quarter (double-buffered)
                xT = xp.tile([128, KD, QW], BF16, tag="xT")
                for g in range(4):
                    xf = xp.tile([128, 2, D], F32, tag="xf")
                    nc.sync.dma_start(xf[:], x_r[:, ds(q * 8 + g * 2, 2), :])
                    xb = xp.tile([128, 2, D], BF16, tag="xb")
                    nc.gpsimd.tensor_copy(xb[:], xf[:])
                    tps = xtp.tile([128, 2, KD, 128], BF16, tag="tps")
                    for j in range(2):
                        for kd in range(KD):
                            nc.tensor.transpose(
                                tps[:, j, kd, :],
                                xb[:, j, ds(kd * 128, 128)], ident_b[:])
                    nc.scalar.activation(
                        xT[:, :, ds(g * 256, 256)],
                        tps[:].rearrange("p j k c -> p k j c"), AF.Copy)
                # Q/K projections, head 0 -> SBUF, head 1 -> DRAM
                for h in range(2):
                    for qi in range(2):
                        for nb in range(2):
                            gb = q * 2 + nb
                            ps = qpp.tile([64, 512], F32, tag="psq")
                            for kd in range(KD):
                                nc.tensor.matmul(
                                    ps[:],
                                    wsb[:, qi, kd, ds(h * 64, 64)],
                                    xT[:, kd, ds(nb * 512, 512)],
                                    start=(kd == 0), stop=(kd == KD - 1))
                            if h == 0:
                                dst = QT8 if qi == 0 else KT8
                                nc.scalar.activation(
                                    dst[:, 0, ds(gb * 512, 512)], ps[:],
                                    AF.Copy)
                            else:
                                stg = qst.tile([64, 512], FP8, tag="stg")
                                nc.scalar.activation(stg[:], ps[:], AF.Copy)
                                nc.sync.dma_start(
                                    qk8_dr[qi, :, ds(gb * 512, 512)], stg[:])
                # V projection + LN stats (two 4-chunk groups per quarter)
                for lg in range(2):
                    cg = q * 2 + lg
                    vps = vpp.tile([128, 4, 128], F32, tag="vps")
                    for j in range(4):
                        for kd in range(KD):
                            nc.tensor.matmul(
                                vps[:, j, :],
                                xT[:, kd, ds((lg * 4 + j) * 128, 128)],
                                wsb[:, 2, kd, :],
                                start=(kd == 0), stop=(kd == KD - 1))
                    vsl = vsb[:, ds(cg * 4, 4), :, :]
                    nc.vector.tensor_copy(
                        vsl, vps[:].rearrange("p j (h d) -> p j h d", h=HPC))
                    nc.vector.tensor_reduce(
                        s1[:].rearrange("p a b -> p (a b)"), vsl, axis=AX.X,
                        op=ALU.add)
                    sqs = vst.tile([128, 4, HPC, 64], BF16, tag="sqs")
                    nc.gpsimd.tensor_tensor(sqs[:], vsl, vsl, ALU.mult)
                    nc.vector.tensor_reduce(
                        s2[:].rearrange("p a b -> p (a b)"), sqs[:], axis=AX.X,
                        op=ALU.add)
                    muf = mu[:, ds(cg * 4, 4)].rearrange(
                        "p a b c -> p (a b c)")
                    varf = var[:, ds(cg * 4, 4)].rearrange(
                        "p a b c -> p (a b c)")
                    s1f = s1[:].rearrange("p a b -> p (a b)")
                    s2f = s2[:].rearrange("p a b -> p (a b)")
                    nc.vector.tensor_scalar_mul(muf, s1f, 1.0 / 64.0)
                    nc.vector.scalar_tensor_tensor(
                        varf, muf, -1.0, muf, ALU.mult, ALU.mult)   # -mu^2
                    nc.vector.scalar_tensor_tensor(
                        varf, s2f, 1.0 / 64.0, varf, ALU.mult, ALU.add)
                    # chain part 1 (rstd-independent): va = (v - mu) * gamma
                    sh = [128, 4, HPC, 64]
                    va = vsb[:, ds(cg * 4, 4), :, :]
                    nc.gpsimd.tensor_tensor(
                        va, va, mu[:, ds(cg * 4, 4)].broadcast_to(sh),
                        ALU.subtract)
                    nc.gpsimd.tensor_tensor(
                        va, va, gbc[:].broadcast_to(sh), ALU.mult)

            def vln_chain_units():
                # rstd = exp(-0.5 * ln(var + eps)), one Ln + one Exp total
                varf = var[:].rearrange("p a b c -> p (a b c)")
                nc.vector.tensor_scalar_add(varf, varf, LN_EPS)
                nc.scalar.activation(varf, varf, AF.Ln)
                nc.scalar.activation(
                    rstd[:].rearrange("p a b c -> p (a b c)"), varf,
                    AF.Exp, scale=-0.5)
                yield
                # part 2: vln = va * rstd + beta, mostly on idle Pool
                for q in range(NC // 4):
                    sh = [128, 4, HPC, 64]
                    va = vsb[:, ds(q * 4, 4), :, :]
                    vsl = vln[:, ds(q * 4, 4), :, :]
                    eng = nc.gpsimd if q % 4 != 3 else nc.vector
                    eng.tensor_tensor(
                        vsl, va, rstd[:, ds(q * 4, 4)].broadcast_to(sh),
                        ALU.mult)
                    eng.tensor_tensor(
                        vsl, vsl, bbc[:].broadcast_to(sh), ALU.add)
                    eng.tensor_copy(vln8[:, ds(q * 4, 4), :, 0:64], vsl)
                    yield
                    # y init for this group: y_h = c0_h * vln_h
                    for h in range(HPC):
                        nc.vector.tensor_scalar_mul(
                            yt[:, ds(q * 4, 4), h, :],
                            vln[:, ds(q * 4, 4), h, :], cbc[:, ds(h * 4, 1)])
                    yield

            emit_quarter(0)
            # phase-A: q-strip 0 exp slots overlap the rest of setup
            slotsA = [mc * N + hh * 512 for mc in range(NC)
                      for hh in range(2)]
            with tc.tile_pool(name="ring0a", bufs=4, space="PSUM") as ring0a:
                gA = prod_gen(ring0a, 512, slotsA, 0,
                              lambda c, w: [(Pflat[:, ds(c, w)], w)])
                done_a = False
                for q in (1, 2, 3):
                    emit_quarter(q)
                    for _ in range(14):
                        if next(gA, "end") == "end":
                            done_a = True
                            break
                gC = vln_chain_units()
                done_c = False
                while not (done_a and done_c):
                    if not done_a:
                        for _ in range(2):
                            if next(gA, "end") == "end":
                                done_a = True
                                break
                    if not done_c:
                        done_c = next(gC, "end") == "end" 


        # head-1 chunk c -> spare slot c (c < STAG) else P slot c - STAG
        def app_lhsT(h, kp, j, P_spare):
            """Stationary P pair (chunks 2kp, 2kp+1) for q-window j."""
            if h == 0:
                return P[:, ds(2 * kp, 2), ds(j * 128, 128)]
            c = 2 * kp
            if c + 2 <= STAG:
                return P_spare[:, ds(c, 2), ds(j * 128, 128)]
            return P[:, ds(c - STAG, 2), ds(j * 128, 128)]

        def h1_pw(P_spare):
            Psf = P_spare[:].rearrange("p c n -> p (c n)")
            sb = STAG * N

            def pw(col, w):
                # [col, col+w) relative to head start; spare covers first sb
                if col + w <= sb:
                    return [(Psf[:, ds(col, w)], w)]
                if col >= sb:
                    return [(Pflat[:, ds(col - sb, w)], w)]
                w0 = sb - col
                return [(Psf[:, ds(col, w0)], w0),
                        (Pflat[:, ds(0, w - w0)], w - w0)]
            return pw

        def apps_gen(h, accp, P_spare, apps=(1, 2, 3), gsz=8):
            """Graph-filter applications for head h (natural-layout form).
            Yields ("grp", app, last_chunk) after each group epilogue."""
            acc_w = 512
            for app in apps:
                if app == 1:
                    groups = [list(range(g * 7, min(NC, g * 7 + 7)))
                              for g in range(5)]
                    W = 65
                else:
                    groups = [list(range(g * gsz, g * gsz + gsz))
                              for g in range(NC // gsz)]
                    W = 64
                for cl in groups:
                    L = len(cl)
                    t8r, t8w = t8s[app % 2], t8s[(app + 1) % 2]
                    acc = accp.tile([128, acc_w], F32, tag="acc")
                    for ji, j in enumerate(cl):
                        for kp in range(NC // 2):
                            rhs = (vln8[:, ds(2 * kp, 2), h, 0:W] if app == 1
                                   else t8r[:, ds(2 * kp, 2), :])
                            nc.tensor.matmul(
                                acc[:, ds(ji * W, W)],
                                app_lhsT(h, kp, j, P_spare), rhs,
                                start=(kp == 0), stop=(kp == NC // 2 - 1),
                                perf_mode=DR)
                        yield
                    accg = acc[:, 0:L * W].rearrange("p (a b) -> p a b", b=W)
                    c0 = cl[0]
                    if app == 1:
                        nc.vector.reciprocal(
                            rinv[:, ds(c0, L), :], accg[:, :, 64:65])
                    # t8 = (u * scale) * (1/r)
                    nc.vector.scalar_tensor_tensor(
                        t8w[:, ds(c0, L), :], accg[:, :, 0:64],
                        T_SCALE if app == 1 else 1.0,
                        rinv[:, ds(c0, L), :].broadcast_to([128, L, 64]),
                        ALU.mult, ALU.mult)
                    if h == 1 and app == 3:
                        # per-group y so outproj can chase app3
                        nc.vector.scalar_tensor_tensor(
                            yt[:, ds(c0, L), h, :], t8w[:, ds(c0, L), :],
                            cbc16[:, ds(h * 4 + app, 1)],
                            yt[:, ds(c0, L), h, :], ALU.mult, ALU.add)
                    yield ("grp", app, cl[-1])
                if not (h == 1 and app == 3):
                    # y += (c_app / T_SCALE) * t8, batched off the group chain
                    nc.vector.scalar_tensor_tensor(
                        yt[:, :, h, :], t8s[(app + 1) % 2][:],
                        cbc16[:, ds(h * 4 + app, 1)],
                        yt[:, :, h, :], ALU.mult, ALU.add)
                yield ("app", app, NC - 1)

        def make_outproj(opp, ost, obt):
            """Output projection in 2-chunk units, stage-skewed emission so
            PE never head-of-line blocks on the yst copy."""
            state = {"ty": 0, "tail": 0, "yst_tiles": {}}

            def emit_ty(u):
                c0 = u * 2
                ty = opp.tile([64, 2, HPC, 128], BF16, tag="ty")
                for j in range(2):
                    for h in range(HPC):
                        nc.tensor.transpose(
                            ty[:, j, h, :], yt[:, c0 + j, h, :], ident_b[:])
                yst = ost.tile([64, 2, HPC, 128], BF16, tag="yst")
                nc.scalar.activation(yst[:], ty[:], AF.Copy)
                state["yst_tiles"][u] = yst

            def emit_tail(u):
                c0 = u * 2
                yst = state["yst_tiles"].pop(u)
                po = opp.tile([128, 2, 256], F32, tag="po")
                for j in range(2):
                    for h in range(HPC):
                        nc.tensor.matmul(
                            po[:, j, :], yst[:, j, h, :],
                            wo_s[:, h, :],
                            start=(h == 0), stop=(h == HPC - 1))
                ob = obt.tile([128, 2, 256], BF16, tag="ob")
                nc.vector.tensor_copy(ob[:], po[:])
                nc.sync.dma_start(
                    bounce_in[:].rearrange("(c p) d -> p c d", p=128)[
                        :, ds(c0, 2), :],
                    ob[:])

            def pump(ready_units):
                while state["ty"] < ready_units:
                    emit_ty(state["ty"])
                    state["ty"] += 1
                    while state["tail"] < state["ty"] - 1:
                        emit_tail(state["tail"])
                        state["tail"] += 1

            def flush():
                pump(NC // 2)
                while state["tail"] < NC // 2:
                    emit_tail(state["tail"])
                    state["tail"] += 1

            return pump, flush, state

        with tc.tile_pool(name="spare", bufs=1) as spare_pool:
            P_spare = spare_pool.tile([128, STAG, N], FP8)

            # ---- head 0 phase-B slots: q-strips 1-3, strip-major ----
            # QT8 strip reloads interleave (strip q's Q cols are only read
            # by strip q's own matmuls); KT8 reloads after its last reader.
            nc.sync.dma_start(QT8[:, 0, 0:SW], qk8_dr[0][:, 0:SW])
            slotsB = [mc * N + q * SW for q in (1, 2, 3) for mc in range(NC)]
            with tc.tile_pool(name="ring0", bufs=4, space="PSUM") as ring0:
                g0 = prod_gen(ring0, SW, slotsB, nA,
                              lambda c, w: [(Pflat[:, ds(c, w)], w)])
                for i, _ in enumerate(g0):
                    if i % NC == NC - 1:
                        q = 1 + i // NC
                        nc.sync.dma_start(QT8[:, 0, ds(q * SW, SW)],
                                          qk8_dr[0][:, ds(q * SW, SW)])
            nc.sync.dma_start(KT8[:, 0, :], qk8_dr[1])

            # ---- head 1 slots (1536-wide ring) + head-0 apps interleaved ----
            with (
                tc.tile_pool(name="ring1", bufs=3, space="PSUM") as ring1,
                tc.tile_pool(name="acc0", bufs=2, space="PSUM") as acc0,
            ):
                slots1 = [i * SW for i in range(n_h1)]
                g1 = prod_gen(ring1, SW, slots1, nA + nB, h1_pw(P_spare))
                ga = apps_gen(0, acc0, P_spare)
                done_p = done_a = False
                while not (done_p and done_a):
                    if not done_p:
                        done_p = next(g1, "end") == "end"
                    if not done_a:
                        for _ in range(8):
                            if next(ga, "end") == "end":
                                done_a = True
                                break

            # ---- tail: head-1 apps + output projection (chasing app3) ----
            with (
                tc.tile_pool(name="acc1", bufs=4, space="PSUM") as acc1,
                tc.tile_pool(name="op_psum", bufs=2, space="PSUM") as opp,
                tc.tile_pool(name="o_st", bufs=2) as ost,
                tc.tile_pool(name="ob_st", bufs=2) as obt,
            ):
                for _ in apps_gen(1, acc1, P_spare, apps=(1, 2)):
                    pass
                pump, flush, op_state = make_outproj(opp, ost, obt)
                out_r = out_d.ap().rearrange("(c p) d -> p c d", p=128)
                bnc_r = bounce_in[:].rearrange("(c p) d -> p c d", p=128)
                out_done = [0]

                def chase_out(units):
                    # out rows 0:1024 = chunks 0:8 = outproj units 0:4
                    if collective:
                        return
                    lim = min(units, 4)
                    while out_done[0] < lim:
                        u = out_done[0]
                        nc.sync.dma_start(out_r[:, ds(u * 2, 2), :],
                                          bnc_r[:, ds(u * 2, 2), :])
                        out_done[0] += 1

                for u in apps_gen(1, acc1, P_spare, apps=(3,), gsz=4):
                    if isinstance(u, tuple) and u[0] == "grp":
                        pump((u[2] + 1) // 2)
                        chase_out(op_state["tail"])
                flush()
                chase_out(4)

        # ---------- ReduceScatter (bf16) over the batch group + output ------
        if not collective:
            # output rows were already DMA'd piecewise by chase_out
            return
        G = len(replica_groups[0])
        bounce_out = dram.tile([N // G, D], BF16)
        nc.gpsimd.collective_compute(
            "ReduceScatter", ALU.add, replica_groups=replica_groups,
            ins=[bounce_in.opt()], outs=[bounce_out.opt()])
        nc.sync.dma_start(out_d.ap(), bounce_out[:])


# ----------------------------------------------------------------------------
# host-side entry point
# ----------------------------------------------------------------------------

_CACHED = {}


def _get_compiled(N=4096, n_cores=8, p_dtype=FP8):
    key = (N, n_cores, p_dtype)
    if key not in _CACHED:
        groups = [list(range(g * 4, g * 4 + 4)) for g in range(2)] \
            if n_cores == 8 else [list(range(n_cores))]
        nc = bacc.Bacc("TRN2", target_bir_lowering=False, debug=False,
                       num_devices=n_cores)
        build_kernel(nc, N=N, replica_groups=groups, p_dtype=p_dtype)
        nc.compile()
        _CACHED[key] = nc
    return _CACHED[key]


def make_in_maps(x, Wq, Wk, Wv, Wo, gamma, beta, coeffs, n_cores=8):
    """Shard full inputs into per-core input maps (batch + head-pair)."""
    x = np.asarray(x, np.float32)
    Wq = np.asarray(Wq, np.float32)
    Wk = np.asarray(Wk, np.float32)
    Wv = np.asarray(Wv, np.float32)
    Wo = np.asarray(Wo, np.float32)
    gamma = np.asarray(gamma, np.float32)
    beta = np.asarray(beta, np.float32)
    coeffs = np.asarray(coeffs, np.float32)
    g2 = np.concatenate([gamma, gamma]).reshape(1, 128).copy()
    b2 = np.concatenate([beta, beta]).reshape(1, 128).copy()
    in_maps = []
    for core in range(n_cores):
        b = core // 4 if n_cores == 8 else 0
        hp = core % 4 if n_cores == 8 else core
        cols = slice(hp * 128, (hp + 1) * 128)
        in_maps.append({
            "x": np.ascontiguousarray(x[b]),
            "wq": np.ascontiguousarray(Wq[:, cols]),
            "wk": np.ascontiguousarray(Wk[:, cols]),
            "wv": np.ascontiguousarray(Wv[:, cols]),
            "wo": np.ascontiguousarray(Wo[cols, :]),
            "gamma2": g2,
            "beta2": b2,
            "cf": np.ascontiguousarray(coeffs[2 * hp: 2 * hp + 2].reshape(1, 8)),
        })
    return in_maps


def kernel(x, Wq, Wk, Wv, Wo, gamma, beta, coeffs, trace=False):
    from concourse.bass_utils import run_bass_kernel_spmd

    n_cores = 8
    nc = _get_compiled(4096, n_cores)
    in_maps = make_in_maps(x, Wq, Wk, Wv, Wo, gamma, beta, coeffs, n_cores)
    res = run_bass_kernel_spmd(nc, in_maps, core_ids=list(range(n_cores)),
                               trace=trace)
    # each core returns its ReduceScatter shard: rank k of a batch group
    # holds rows [k*N/4, (k+1)*N/4) of that batch's output
    N = 4096
    out = np.empty((2, N, 256), np.float32)
    for b, cores in enumerate([[0, 1, 2, 3], [4, 5, 6, 7]]):
        for k, c in enumerate(cores):
            shard = np.asarray(res.results[c]["out"]).astype(np.float32)
            out[b, k * (N // 4):(k + 1) * (N // 4)] = shard
    if trace:
        kernel.last_result = res
    return out
